# revision 20
# baseline (speedup 1.0000x reference)
"""Trainium2 Bass kernel for MemoryOptimizedAttention (MHA with projections).

Problem (hardcoded): B=4, T=2048, D=1024, H=16, DH=64, fp32 I/O.

Sharding: 8 cores = (batch b, head-half h) pairs — data parallel on B,
tensor parallel on heads. Each core projects only its 8 heads' Q/K/V
features (columns of Wq/Wk/Wv) for the full 2048-token context — no
cross-core duplication — runs attention for those heads over all 2048
queries, and applies its half of Wo (rows), producing a partial output.
The host gather adds the two partials per batch. No collectives.

Device dataflow (feature-major / transposed layouts throughout):
  QT = Wq_h @ xqT (+bq_h)        [512, 2048]    per head-pair (hp) chunks
  KT = Wk_h @ xkT (+bk_h)        [512, 2048]
  V' = xvT.T @ Wv_hT             [2048, 130/hp] k-major, ones cols at 64/129
  S^T[k,q] = KT_p.T-slice @ QT_p [128, 2, 512]  row-tiled pair (concurrent)
  O'^T = V'^T @ attn^T           [65, 512] per head (denom rides at row 64)
  O^T[p,q] = O' * bcast(1/denom) [128, 4, 2048]  A rows 0-63, B rows 64-127
  Y^T_partial = Wo_h @ O^T       [1024, 2048]   128-deep contraction chunks
bv and bo are folded on the host into a constant row added to the output.

Scheduling: the S->exp stream runs CONTINUOUSLY at one (t,kc) round per
slot across q-tile, head-pair, and repeat-boundaries, so the ScalarE exp
pipe never starves. AV consumption lags the S stream by L0 slots in steady
state; at each q-tile boundary the lag stretches to LB slots (catching up
two AVs per slot afterwards) so the avp PSUM-bank WAR against the previous
tile's normalize chain gets a ~LB-slot window. The normalize chain uses
reciprocal_approx_fast (~0.7us vs 4us for full reciprocal; 18 bits is
plenty for a softmax denominator) and is emission-ordered so head B's
reciprocal overlaps head A's gpsimd broadcast. Projections for the next
head pair interleave into the current pair's attention slots; the final
q-tile's output-projection groups carry into the next repeat.
"""

import sys

for _p in ("/opt/trn_rl_repo",):
    if _p not in sys.path:
        sys.path.insert(0, _p)

import numpy as np

import concourse.bass as bass
import concourse.mybir as mybir
import concourse.tile as tile
from concourse import bacc
from concourse import bass_utils
from concourse.bass import ts, ds

B, T, D, H = 4, 2048, 1024, 16
DH = D // H
SCALE = 1.0 / float(np.sqrt(DH))

P = 128
HP = 4     # head pairs per core (8 heads)
CC = 8     # 128-wide chunks of D (projection contraction)
KC = 16    # 128-wide chunks of the key/context dim (2048)
QT = 4     # 512-wide q tiles per core (full T)
F = 512
TQ = 2048  # q rows per core
TK = 2048  # context rows per core

L0 = 4     # steady AV lag behind the S stream (slots)
LB = 9     # stretched lag at q-tile boundaries (avp WAR window)

fp16 = mybir.dt.float16
f32 = mybir.dt.float32
EXP = mybir.ActivationFunctionType.Exp

N_CORES = 8
NROUND = HP * QT * KC  # 256 attention rounds per body


def _interleave(a_items, b_items):
    """Emit two work-item lists interleaved evenly."""
    na, nb = len(a_items), len(b_items)
    ia = ib = 0
    while ia < na or ib < nb:
        if ia >= na:
            b_items[ib]()
            ib += 1
        elif ib >= nb:
            a_items[ia]()
            ia += 1
        elif ib * na <= ia * nb:
            b_items[ib]()
            ib += 1
        else:
            a_items[ia]()
            ia += 1


def _av_schedule():
    """slot index for each AV round: lag L0 behind its S slot, stretched to
    LB at q-tile boundaries, monotone, at most 2 AV emissions per slot."""
    slots = []
    last = -1
    per_slot = {}
    for a in range(NROUND):
        need = LB if (a % KC == 0 and a > 0) else L0
        s = max(last, a + need)
        while per_slot.get(s, 0) >= 2:
            s += 1
        per_slot[s] = per_slot.get(s, 0) + 1
        slots.append(s)
        last = s
    return slots


def build_nc(repeat=1):
    nc = bacc.Bacc(None, target_bir_lowering=False, debug=False)

    xq = nc.dram_tensor("xq", [P, CC, TQ], fp16, kind="ExternalInput")
    xk = nc.dram_tensor("xk", [P, CC, TK], fp16, kind="ExternalInput")
    xv = nc.dram_tensor("xv", [P, CC, TK], fp16, kind="ExternalInput")
    wq = nc.dram_tensor("wq", [P, HP, CC, P], fp16, kind="ExternalInput")
    wk = nc.dram_tensor("wk", [P, HP, CC, P], fp16, kind="ExternalInput")
    wv = nc.dram_tensor("wv", [P, CC, HP * P], fp16, kind="ExternalInput")
    wo = nc.dram_tensor("wo", [P, CC, HP, P], fp16, kind="ExternalInput")
    bq = nc.dram_tensor("bq", [P, HP], f32, kind="ExternalInput")
    bk = nc.dram_tensor("bk", [P, HP], f32, kind="ExternalInput")
    yT = nc.dram_tensor("yT", [P, CC, TQ], f32, kind="ExternalOutput")

    with tile.TileContext(nc) as tc:
        with (
            tc.tile_pool(name="res", bufs=1) as res,
            tc.tile_pool(name="wpool", bufs=2) as wpool,
            tc.tile_pool(name="hpp", bufs=2) as hpp,
            tc.tile_pool(name="apool", bufs=LB + 2) as apool,
            tc.tile_pool(name="npool", bufs=2) as npool,
            tc.tile_pool(name="mmp", bufs=1, space="PSUM") as mmp,
            tc.tile_pool(name="stp", bufs=2, space="PSUM") as stp,
            tc.tile_pool(name="avp", bufs=2, space="PSUM") as avp,
            tc.tile_pool(name="bcp", bufs=1, space="PSUM") as bcp,
        ):
            bq_sb = res.tile([P, HP], f32)
            bk_sb = res.tile([P, HP], f32)
            # resident x tiles; DMAs are emitted after the first head-pair's
            # weight loads (see below) so the first projections start early
            xq_sb = res.tile([P, CC, TQ], fp16)
            xk_sb = res.tile([P, CC, TK], fp16)
            xv_sb = res.tile([P, CC, TK], fp16)
            wv_sb = res.tile([P, CC, HP * P], fp16)
            wo_sb = res.tile([P, CC, HP, P], fp16)
            # V' for this core's head pairs, k-major, with ones cols at
            # 64/129; one tile per k-chunk keeps access patterns simple for
            # the dependency tracker
            vp_sbs = [
                res.tile([P, HP, 130], fp16, name=f"vp_sb{k}") for k in range(KC)
            ]

            def load_x_head():
                # the first Q-projection matmuls gate the whole dispatch, so
                # their xq chunks go right behind wq at the head of the SP
                # queue, finest-grained first
                nc.sync.dma_start(xq_sb[:, 0, ts(0, F)], xq[:, 0, ts(0, F)])
                nc.sync.dma_start(xq_sb[:, 1, ts(0, F)], xq[:, 1, ts(0, F)])
                for g in range(1, 4):
                    nc.sync.dma_start(
                        xq_sb[:, 2 * g : 2 * g + 2, ts(0, F)],
                        xq[:, 2 * g : 2 * g + 2, ts(0, F)],
                    )

            def load_x():
                # slab DMAs aligned with first consumers, issued from three
                # sequencers in parallel so descriptor generation does not
                # serialize the startup: SP feeds Q, Act feeds K and wv,
                # Pool (SWDGE) feeds the early xv k-chunks
                nc.sync.dma_start(bq_sb[:], bq[:])
                nc.sync.dma_start(bk_sb[:], bk[:])
                nc.scalar.dma_start(xk_sb[:, 0:4, ts(0, F)], xk[:, 0:4, ts(0, F)])
                nc.scalar.dma_start(xk_sb[:, 4:8, ts(0, F)], xk[:, 4:8, ts(0, F)])
                for t in range(1, TK // F):
                    nc.scalar.dma_start(xk_sb[:, :, ts(t, F)], xk[:, :, ts(t, F)])
                nc.gpsimd.dma_start(wv_sb[:], wv[:])
                for kc in range(KC // 2):
                    nc.gpsimd.dma_start(xv_sb[:, :, ts(kc, P)], xv[:, :, ts(kc, P)])
                for t in range(1, QT):
                    nc.sync.dma_start(xq_sb[:, :, ts(t, F)], xq[:, :, ts(t, F)])
                for kc in range(KC // 2, KC):
                    nc.sync.dma_start(xv_sb[:, :, ts(kc, P)], xv[:, :, ts(kc, P)])
                for dc in range(CC):
                    nc.sync.dma_start(wo_sb[:, dc], wo[:, dc])
                for k in range(KC):
                    nc.vector.memset(vp_sbs[k][:, :, 64:65], 1.0)
                    nc.vector.memset(vp_sbs[k][:, :, 129:130], 1.0)

            # O^T, head-pair packed: A dims at partitions 0-63, B at 64-127
            ot_sb = res.tile([P, HP, TQ], fp16)

            # proj-group PSUM ping-pong across mmp/bcp to dodge the
            # single-buffer eviction WAR; positional counter spans all
            # projection/output groups
            pp = [0]

            def proj_ps():
                pool = (mmp, bcp)[pp[0] % 2]
                pp[0] += 1
                return pool.tile(
                    [P, F], f32, tag="proj" if pool is mmp else "bps", name="ps"
                )

            def vp_group(kc):
                # V' projection for all 4 head pairs at once (512 out dims)
                ps = proj_ps()
                for c in range(CC):
                    nc.tensor.matmul(
                        ps[:],
                        xv_sb[:, c, ts(kc, P)],
                        wv_sb[:, c, :],
                        start=(c == 0),
                        stop=(c == CC - 1),
                    )
                ps4 = ps[:].rearrange("p (hp g j) -> p hp g j", hp=HP, g=2)
                vp4 = vp_sbs[kc][:].rearrange("p hp (g x) -> p hp g x", g=2)
                nc.vector.tensor_copy(vp4[:, :, :, 0:64], ps4[:])

            states = {}

            def proj_items(hp):
                state = states.setdefault(hp, {})

                def dma_wq():
                    # halved weight DMAs: the first projection matmul only
                    # needs chunk 0, so it starts as soon as the first half
                    # lands
                    wq_t = wpool.tile([P, CC, P], fp16, tag="wq", name="wq_t")
                    nc.sync.dma_start(wq_t[:, 0:4], wq[:, hp, 0:4])
                    nc.sync.dma_start(wq_t[:, 4:8], wq[:, hp, 4:8])
                    qt_sb = hpp.tile([P, TQ], fp16, tag="qt", name="qt_sb")
                    kt_sb = hpp.tile([P, TK], fp16, tag="kt", name="kt_sb")
                    state.update(wq_t=wq_t, qt_sb=qt_sb, kt_sb=kt_sb)

                def dma_wk():
                    wk_t = wpool.tile([P, CC, P], fp16, tag="wk", name="wk_t")
                    nc.sync.dma_start(wk_t[:, 0:4], wk[:, hp, 0:4])
                    nc.sync.dma_start(wk_t[:, 4:8], wk[:, hp, 4:8])
                    state.update(wk_t=wk_t)

                def dma_weights():
                    dma_wq()
                    dma_wk()

                def qt_group(t):
                    ps = proj_ps()
                    for c in range(CC):
                        nc.tensor.matmul(
                            ps[:],
                            state["wq_t"][:, c, :],
                            xq_sb[:, c, ts(t, F)],
                            start=(c == 0),
                            stop=(c == CC - 1),
                        )
                    nc.vector.tensor_scalar_add(
                        state["qt_sb"][:, ts(t, F)],
                        ps[:],
                        bq_sb[:, hp : hp + 1],
                    )

                def kt_group(t):
                    ps = proj_ps()
                    for c in range(CC):
                        nc.tensor.matmul(
                            ps[:],
                            state["wk_t"][:, c, :],
                            xk_sb[:, c, ts(t, F)],
                            start=(c == 0),
                            stop=(c == CC - 1),
                        )
                    nc.vector.tensor_scalar_add(
                        state["kt_sb"][:, ts(t, F)],
                        ps[:],
                        bk_sb[:, hp : hp + 1],
                    )

                items = [dma_weights, lambda: qt_group(0)]
                for t in range(TK // F):
                    items.append(lambda t=t: kt_group(t))
                for t in range(1, QT):
                    items.append(lambda t=t: qt_group(t))
                return state, items, (dma_wq, dma_wk, qt_group, kt_group)

            # ---- global attention stream ----------------------------------
            def s_item(hp, t, kc):
                state = states[hp]
                qt_sb, kt_sb = state["qt_sb"], state["kt_sb"]
                st = stp.tile([P, 2, F], f32, tag="st", name="st")
                # head-pair packed: A rows 0-63, B rows 64-127; the two
                # 64-contraction matmuls row-tile onto disjoint row groups
                # and run concurrently (HW-verified ~one 512-col pass total)
                nc.tensor.matmul(
                    st[:, 0, :],
                    kt_sb[0:DH, ts(kc, P)],
                    qt_sb[0:DH, ts(t, F)],
                    start=True,
                    stop=True,
                )
                nc.tensor.matmul(
                    st[:, 1, :],
                    kt_sb[DH:P, ts(kc, P)],
                    qt_sb[DH:P, ts(t, F)],
                    start=True,
                    stop=True,
                )
                at = apool.tile([P, 2, F], fp16, tag="attn", name="at")
                nc.scalar.activation(at[:], st[:], EXP, scale=SCALE)
                state[("at", t, kc)] = at

            def av_item(hp, t, kc):
                state = states[hp]
                if kc == 0:
                    state[("avA", t)] = avp.tile([P, F], f32, tag="av", name="avA")
                    state[("avB", t)] = avp.tile([P, F], f32, tag="av", name="avB")
                at = state.pop(("at", t, kc))
                avA, avB = state[("avA", t)], state[("avB", t)]
                nc.tensor.matmul(
                    avA[0:65, :],
                    vp_sbs[kc][:, hp, 0:65],
                    at[:, 0, :],
                    start=(kc == 0),
                    stop=(kc == KC - 1),
                )
                nc.tensor.matmul(
                    avB[0:65, :],
                    vp_sbs[kc][:, hp, 65:130],
                    at[:, 1, :],
                    start=(kc == 0),
                    stop=(kc == KC - 1),
                )

            def norm_pre(hp, t):
                # both av psums carry their denominator at row 64; approx
                # reciprocal (18 bits, ample for a softmax denom) on DVE,
                # fp16 convert, then gpsimd broadcast across the head's 64
                # partitions. Emission order lets head B's recip/convert
                # overlap head A's broadcast on the Pool engine.
                state = states[hp]
                bcs = []
                for h in range(2):
                    av = state[("avA", t)] if h == 0 else state[("avB", t)]
                    # shift the denominator row 64 -> 0 with a plain copy
                    # (32-aligned partition shift, HW-proven); the custom
                    # reciprocal op itself cannot shift lanes
                    dn = npool.tile([P, F], f32, tag="dn", name="dn")
                    nc.vector.tensor_copy(dn[0:1, :], av[64:65, :])
                    rdf = npool.tile([P, F], f32, tag="rdf", name="rdf")
                    nc.vector.reciprocal_approx_fast(rdf[0:1, :], dn[0:1, :])
                    rd = npool.tile([P, F], fp16, tag="rd", name="rd")
                    nc.vector.tensor_copy(rd[0:1, :], rdf[0:1, :])
                    bc = npool.tile([P, F], fp16, tag="bc", name="bc")
                    nc.gpsimd.partition_broadcast(bc[0:DH, :], rd[0:1, :])
                    bcs.append(bc)
                state[("bc", t)] = tuple(bcs)

            def norm_mul(hp, t, h):
                state = states[hp]
                av = state[("avA", t)] if h == 0 else state[("avB", t)]
                bc = state[("bc", t)][h]
                rows = slice(0, DH) if h == 0 else slice(DH, P)
                nc.vector.tensor_mul(
                    ot_sb[rows, hp, ts(t, F)], av[0:DH, :], bc[0:DH, :]
                )

            av_slot = _av_schedule()

            def stream_slots():
                """One closure per S slot; each emits its S round, any AV
                rounds scheduled for the slot, and normalize items due."""
                by_slot = {}
                for a, s in enumerate(av_slot):
                    by_slot.setdefault(s, []).append(a)

                def make(g):
                    def run():
                        hp_s, t_s, kc_s = g // 64, (g // KC) % QT, g % KC
                        s_item(hp_s, t_s, kc_s)
                        for a in by_slot.get(g, ()):
                            hp_a, t_a, kc_a = a // 64, (a // KC) % QT, a % KC
                            av_item(hp_a, t_a, kc_a)
                            if kc_a == KC - 1:
                                norm_pre(hp_a, t_a)
                                norm_mul(hp_a, t_a, 0)
                                norm_mul(hp_a, t_a, 1)
                    return run

                slots = [make(g) for g in range(NROUND)]
                tail = []

                def make_tail(a):
                    def run():
                        hp_a, t_a, kc_a = a // 64, (a // KC) % QT, a % KC
                        av_item(hp_a, t_a, kc_a)
                        if kc_a == KC - 1:
                            norm_pre(hp_a, t_a)
                            norm_mul(hp_a, t_a, 0)
                            norm_mul(hp_a, t_a, 1)
                    return run

                for a, s in enumerate(av_slot):
                    if s >= NROUND:
                        tail.append(make_tail(a))
                return slots, tail

            def y_items():
                def y_group_t(dc, t):
                    ysb = npool.tile([P, F], f32, tag="y", name="ysb", bufs=2)
                    ps = proj_ps()
                    for s in range(HP):
                        nc.tensor.matmul(
                            ps[:],
                            wo_sb[:, dc, s, :],
                            ot_sb[:, s, ts(t, F)],
                            start=(s == 0),
                            stop=(s == HP - 1),
                        )
                    if t == QT - 1:
                        # final q-tile: halve the evict+store so the last
                        # write-out pipelines instead of trailing whole, and
                        # issue the stores from the Act/Pool sequencers —
                        # both idle by now — so descriptor generation does
                        # not serialize on SP behind the last matmuls
                        engines = [nc.scalar, nc.gpsimd]
                        for g in range(2):
                            h = ds(g * (F // 2), F // 2)
                            nc.vector.tensor_copy(ysb[:, h], ps[:, h])
                            engines[(dc + g) % 2].dma_start(
                                yT[:, dc, ds(t * F + g * (F // 2), F // 2)], ysb[:, h]
                            )
                    else:
                        nc.vector.tensor_copy(ysb[:], ps[:])
                        nc.sync.dma_start(yT[:, dc, ts(t, F)], ysb[:])

                # emission-order safety: a read emitted before its writer
                # gets no RAW edge, so a Y group may only be emitted after
                # ALL normalize items writing the ot slices it reads.
                return [
                    [lambda dc=dc, t=t: y_group_t(dc, t) for dc in range(CC)]
                    for t in range(QT)
                ]

            pending = []
            for _rep in range(repeat):
                slots, tail = stream_slots()
                for hp in range(HP):
                    state, pitems, raw = proj_items(hp)
                    if _rep == 0 and hp == 0:
                        # hand-rolled first phase: weights + x DMAs, then
                        # Q/K groups and V' groups threaded through the t0
                        # slots they feed — kt(g) lands just before its
                        # first S consumer and V'(kc) at slot kc, L0 slots
                        # ahead of its AV consumer, so the exp pipeline
                        # starts as early as possible.
                        dma_wq, dma_wk, qt_g, kt_g = raw
                        dma_wq()
                        load_x_head()
                        dma_wk()
                        load_x()
                        qt_g(0)
                        kt_g(0)
                        for kc in range(KC):
                            if kc % 4 == 0 and kc > 0:
                                kt_g(kc // 4)
                            vp_group(kc)
                            slots[kc]()
                        for t in range(1, QT):
                            qt_g(t)
                        pending = slots[KC:64]
                        continue
                    if hp == 0:
                        # rep > 0: the V' projection is part of every body;
                        # thread its 16 groups into the hp0 proj phase
                        # (their first AV consumers run L0+ slots later)
                        pitems = (
                            pitems[:6]
                            + [lambda kc=kc: vp_group(kc) for kc in range(KC)]
                            + pitems[6:]
                        )
                    _interleave(pending, pitems)
                    pending = slots[hp * 64 : (hp + 1) * 64]
                # Y groups for q-tile t interleave with the hp3 slots after
                # hp3's normalize(t) (whose emission slot follows from the
                # AV schedule); the final q-tile's groups and the AV tail
                # carry into the next repeat's projection phase.
                y_gs = y_items()
                hp3 = pending
                cuts = [
                    min(av_slot[3 * 64 + KC * t + KC - 1] - 3 * 64 + 1, 64)
                    for t in range(QT)
                ]
                _interleave(hp3[: cuts[0]], [])
                seg_prev = cuts[0]
                for t in range(QT - 1):
                    seg_end = cuts[t + 1] if t < QT - 2 else 64
                    _interleave(hp3[seg_prev:seg_end], y_gs[t])
                    seg_prev = seg_end
                pending = tail + y_gs[QT - 1]
            for it in pending:
                it()

    nc.compile()
    return nc


_NC_CACHE = None


def _get_nc():
    global _NC_CACHE
    if _NC_CACHE is None:
        _NC_CACHE = build_nc()
    return _NC_CACHE


def _chunk_T(a):
    # [rows, D] f32 -> [P, D//P, rows] fp16 (feature-major chunks)
    return np.ascontiguousarray(
        a.T.reshape(CC, P, a.shape[0]).transpose(1, 0, 2).astype(np.float16)
    )


def _prep_in_maps(query, key, value, Wq, bq, Wk, bk, Wv, bv, Wo, bo):
    HD = HP * P  # 512 output dims per core

    def _shard_w_out(W, h):
        # rows [h*512, (h+1)*512) of a [D, D] torch-Linear weight, laid out
        # [p, hp, c, j] = W[h*512 + hp*128 + p, 128c + j]
        Wh = W[h * HD : (h + 1) * HD, :]
        return np.ascontiguousarray(
            Wh.reshape(HP, P, CC, P).transpose(3, 0, 2, 1).astype(np.float16)
        )

    wv_d, wo_d, bq_d, bk_d = [], [], [], []
    for h in range(2):
        Wvh = Wv[h * HD : (h + 1) * HD, :]
        # wv: [p, c, j] = Wvh[j, 128c + p] (dout-major for 512-wide groups)
        wv_d.append(
            np.ascontiguousarray(
                Wvh.reshape(HD, CC, P).transpose(2, 1, 0).astype(np.float16)
            )
        )
        # wo: [p2, dc, hp, j] = Wo[dc*128+j, h*512 + hp*128 + p2], p2 = a*64+dh
        Woh = Wo[:, h * HD : (h + 1) * HD]
        wo_d.append(
            np.ascontiguousarray(
                Woh.reshape(CC, P, HP, 2, DH)
                .transpose(3, 4, 0, 2, 1)
                .reshape(P, CC, HP, P)
                .astype(np.float16)
            )
        )
        bq_d.append(
            np.ascontiguousarray(
                bq[h * HD : (h + 1) * HD].reshape(HP, P).T.astype(np.float32)
            )
        )
        bk_d.append(
            np.ascontiguousarray(
                bk[h * HD : (h + 1) * HD].reshape(HP, P).T.astype(np.float32)
            )
        )

    xq_b = [_chunk_T(query[b]) for b in range(B)]
    xk_b = [_chunk_T(key[b]) for b in range(B)]
    xv_b = [_chunk_T(value[b]) for b in range(B)]

    in_maps = []
    for c in range(N_CORES):
        b, h = divmod(c, 2)
        in_maps.append(
            {
                "xq": xq_b[b],
                "xk": xk_b[b],
                "xv": xv_b[b],
                "wq": _shard_w_out(Wq, h),
                "wk": _shard_w_out(Wk, h),
                "wv": wv_d[h],
                "wo": wo_d[h],
                "bq": bq_d[h],
                "bk": bk_d[h],
            }
        )
    return in_maps


def _gather(results, bv, bo, Wo):
    crow = (bv.astype(np.float32) @ Wo.T.astype(np.float32) + bo).astype(np.float32)
    out = np.empty((B, T, D), np.float32)
    for b in range(B):
        # two head-half partials per batch; sum them
        y0 = results[2 * b]["yT"]  # [P, CC, TQ] f32 = Y^T partial chunks
        y1 = results[2 * b + 1]["yT"]
        y = (y0 + y1).transpose(1, 0, 2).reshape(D, TQ).T
        out[b] = y + crow
    return out


def _run(inputs, trace=False, **kwargs):
    inputs = {k: np.asarray(v) for k, v in inputs.items()}
    nc = _get_nc()
    in_maps = _prep_in_maps(**inputs)
    res = bass_utils.run_bass_kernel_spmd(
        nc, in_maps, core_ids=list(range(N_CORES)), trace=trace, **kwargs
    )
    out = _gather(res.results, inputs["bv"], inputs["bo"], inputs["Wo"])
    return out, res


def kernel(**inputs):
    out, _ = _run(inputs, trace=False)
    return out


# revision 24
# speedup vs baseline: 1.0533x; 1.0533x over previous
"""Trainium2 Bass kernel for MemoryOptimizedAttention (MHA with projections).

Problem (hardcoded): B=4, T=2048, D=1024, H=16, DH=64, fp32 I/O.

Sharding: 8 cores = (batch b, head-half h) pairs — data parallel on B,
tensor parallel on heads. Each core projects only its 8 heads' Q/K/V
features (columns of Wq/Wk/Wv) for the full 2048-token context — no
cross-core duplication — runs attention for those heads over all 2048
queries, and applies its half of Wo (rows), producing a partial output.
The host gather adds the two partials per batch. No collectives.

Device dataflow (feature-major / transposed layouts throughout):
  QT = Wq_h @ xqT (+bq_h)        [512, 2048]    per head-pair (hp) chunks
  KT = Wk_h @ xkT (+bk_h)        [512, 2048]
  V' = xvT.T @ Wv_hT             [2048, 130/hp] k-major, ones cols at 64/129
  S^T[k,q] = KT_p.T-slice @ QT_p [128, 2, 512]  row-tiled pair (concurrent)
  O'^T = V'^T @ attn^T           [65, 512] per head (denom rides at row 64)
  O^T[p,q] = O' * bcast(1/denom) [128, 4, 2048]  A rows 0-63, B rows 64-127
  Y^T_partial = Wo_h @ O^T       [1024, 2048]   128-deep contraction chunks
bv and bo are folded on the host into a constant row added to the output.

Scheduling: the S->exp stream runs CONTINUOUSLY at one (t,kc) round per
slot across q-tile, head-pair, and repeat-boundaries, so the ScalarE exp
pipe never starves. AV consumption lags the S stream by L0 slots in steady
state; at each q-tile boundary the lag stretches to LB slots (catching up
two AVs per slot afterwards) so the avp PSUM-bank WAR against the previous
tile's normalize chain gets a ~LB-slot window. The normalize chain uses
reciprocal_approx_fast (~0.7us vs 4us for full reciprocal; 18 bits is
plenty for a softmax denominator) and is emission-ordered so head B's
reciprocal overlaps head A's gpsimd broadcast. Projections for the next
head pair interleave into the current pair's attention slots; the final
q-tile's output-projection groups carry into the next repeat.
"""

import sys

for _p in ("/opt/trn_rl_repo",):
    if _p not in sys.path:
        sys.path.insert(0, _p)

import numpy as np

import concourse.bass as bass
import concourse.mybir as mybir
import concourse.tile as tile
from concourse import bacc
from concourse import bass_utils
from concourse.bass import ts, ds

B, T, D, H = 4, 2048, 1024, 16
DH = D // H
SCALE = 1.0 / float(np.sqrt(DH))

P = 128
HP = 4     # head pairs per core (8 heads)
CC = 8     # 128-wide chunks of D (projection contraction)
KC = 16    # 128-wide chunks of the key/context dim (2048)
QT = 4     # 512-wide q tiles per core (full T)
F = 512
TQ = 2048  # q rows per core
TK = 2048  # context rows per core

L0 = 4     # steady AV lag behind the S stream (slots)
LB = 8     # stretched lag at q-tile boundaries (avp WAR window)

fp16 = mybir.dt.float16
f32 = mybir.dt.float32
EXP = mybir.ActivationFunctionType.Exp

N_CORES = 8
NROUND = HP * QT * KC  # 256 attention rounds per body


def _interleave(a_items, b_items):
    """Emit two work-item lists interleaved evenly."""
    na, nb = len(a_items), len(b_items)
    ia = ib = 0
    while ia < na or ib < nb:
        if ia >= na:
            b_items[ib]()
            ib += 1
        elif ib >= nb:
            a_items[ia]()
            ia += 1
        elif ib * na <= ia * nb:
            b_items[ib]()
            ib += 1
        else:
            a_items[ia]()
            ia += 1


def _av_schedule():
    """slot index for each AV round: lag L0 behind its S slot, stretched to
    LB at q-tile boundaries, monotone, at most 2 AV emissions per slot."""
    slots = []
    last = -1
    per_slot = {}
    for a in range(NROUND):
        need = LB if (a % KC == 0 and a > 0) else L0
        s = max(last, a + need)
        while per_slot.get(s, 0) >= 2:
            s += 1
        per_slot[s] = per_slot.get(s, 0) + 1
        slots.append(s)
        last = s
    return slots


def build_nc(repeat=1):
    nc = bacc.Bacc(None, target_bir_lowering=False, debug=False)

    xq = nc.dram_tensor("xq", [P, CC, TQ], fp16, kind="ExternalInput")
    xk = nc.dram_tensor("xk", [P, CC, TK], fp16, kind="ExternalInput")
    xv = nc.dram_tensor("xv", [P, CC, TK], fp16, kind="ExternalInput")
    wq = nc.dram_tensor("wq", [P, HP, CC, P], fp16, kind="ExternalInput")
    wk = nc.dram_tensor("wk", [P, HP, CC, P], fp16, kind="ExternalInput")
    wv = nc.dram_tensor("wv", [P, CC, HP * P], fp16, kind="ExternalInput")
    wo = nc.dram_tensor("wo", [P, CC, HP, P], fp16, kind="ExternalInput")
    bq = nc.dram_tensor("bq", [P, HP], f32, kind="ExternalInput")
    bk = nc.dram_tensor("bk", [P, HP], f32, kind="ExternalInput")
    yT = nc.dram_tensor("yT", [P, CC, TQ], f32, kind="ExternalOutput")

    with tile.TileContext(nc) as tc:
        with (
            tc.tile_pool(name="res", bufs=1) as res,
            tc.tile_pool(name="wpool", bufs=2) as wpool,
            tc.tile_pool(name="hpp", bufs=2) as hpp,
            tc.tile_pool(name="apool", bufs=LB + 3) as apool,
            tc.tile_pool(name="npool", bufs=2) as npool,
            tc.tile_pool(name="mmp", bufs=1, space="PSUM") as mmp,
            tc.tile_pool(name="stp", bufs=2, space="PSUM") as stp,
            tc.tile_pool(name="avp", bufs=2, space="PSUM") as avp,
            tc.tile_pool(name="bcp", bufs=1, space="PSUM") as bcp,
        ):
            bq_sb = res.tile([P, HP], f32)
            bk_sb = res.tile([P, HP], f32)
            # resident x tiles; DMAs are emitted after the first head-pair's
            # weight loads (see below) so the first projections start early
            xq_sb = res.tile([P, CC, TQ], fp16)
            xk_sb = res.tile([P, CC, TK], fp16)
            xv_sb = res.tile([P, CC, TK], fp16)
            wv_sb = res.tile([P, CC, HP * P], fp16)
            wo_sb = res.tile([P, CC, HP, P], fp16)
            # V' for this core's head pairs, k-major, with ones cols at
            # 64/129; one tile per k-chunk keeps access patterns simple for
            # the dependency tracker
            vp_sbs = [
                res.tile([P, HP, 130], fp16, name=f"vp_sb{k}") for k in range(KC)
            ]

            def load_x_head():
                # the first Q-projection matmuls gate the whole dispatch, so
                # their xq chunks go right behind wq at the head of the SP
                # queue, finest-grained first
                nc.sync.dma_start(xq_sb[:, 0, ts(0, F)], xq[:, 0, ts(0, F)])
                nc.sync.dma_start(xq_sb[:, 1, ts(0, F)], xq[:, 1, ts(0, F)])
                for g in range(1, 4):
                    nc.sync.dma_start(
                        xq_sb[:, 2 * g : 2 * g + 2, ts(0, F)],
                        xq[:, 2 * g : 2 * g + 2, ts(0, F)],
                    )

            def load_x():
                # slab DMAs aligned with first consumers, issued from three
                # sequencers in parallel so descriptor generation does not
                # serialize the startup: SP feeds Q, Act feeds K and wv,
                # Pool (SWDGE) feeds the early xv k-chunks
                nc.sync.dma_start(bq_sb[:], bq[:])
                nc.sync.dma_start(bk_sb[:], bk[:])
                nc.scalar.dma_start(xk_sb[:, 0:4, ts(0, F)], xk[:, 0:4, ts(0, F)])
                nc.scalar.dma_start(xk_sb[:, 4:8, ts(0, F)], xk[:, 4:8, ts(0, F)])
                for t in range(1, TK // F):
                    nc.scalar.dma_start(xk_sb[:, :, ts(t, F)], xk[:, :, ts(t, F)])
                nc.gpsimd.dma_start(wv_sb[:], wv[:])
                for kc in range(KC // 2):
                    nc.gpsimd.dma_start(xv_sb[:, :, ts(kc, P)], xv[:, :, ts(kc, P)])
                for t in range(1, QT):
                    nc.sync.dma_start(xq_sb[:, :, ts(t, F)], xq[:, :, ts(t, F)])
                for kc in range(KC // 2, KC):
                    nc.sync.dma_start(xv_sb[:, :, ts(kc, P)], xv[:, :, ts(kc, P)])
                for dc in range(CC):
                    nc.sync.dma_start(wo_sb[:, dc], wo[:, dc])
                for k in range(KC):
                    nc.vector.memset(vp_sbs[k][:, :, 64:65], 1.0)
                    nc.vector.memset(vp_sbs[k][:, :, 129:130], 1.0)

            # O^T, head-pair packed: A dims at partitions 0-63, B at 64-127
            ot_sb = res.tile([P, HP, TQ], fp16)

            # proj-group PSUM ping-pong across mmp/bcp to dodge the
            # single-buffer eviction WAR; positional counter spans all
            # projection/output groups
            pp = [0]

            def proj_ps():
                pool = (mmp, bcp)[pp[0] % 2]
                pp[0] += 1
                return pool.tile(
                    [P, F], f32, tag="proj" if pool is mmp else "bps", name="ps"
                )

            def vp_group(kc):
                # V' projection for all 4 head pairs at once (512 out dims)
                ps = proj_ps()
                for c in range(CC):
                    nc.tensor.matmul(
                        ps[:],
                        xv_sb[:, c, ts(kc, P)],
                        wv_sb[:, c, :],
                        start=(c == 0),
                        stop=(c == CC - 1),
                    )
                ps4 = ps[:].rearrange("p (hp g j) -> p hp g j", hp=HP, g=2)
                vp4 = vp_sbs[kc][:].rearrange("p hp (g x) -> p hp g x", g=2)
                nc.vector.tensor_copy(vp4[:, :, :, 0:64], ps4[:])

            states = {}

            def proj_items(hp):
                state = states.setdefault(hp, {})

                def dma_wq():
                    # halved weight DMAs: the first projection matmul only
                    # needs chunk 0, so it starts as soon as the first half
                    # lands
                    wq_t = wpool.tile([P, CC, P], fp16, tag="wq", name="wq_t")
                    nc.sync.dma_start(wq_t[:, 0:4], wq[:, hp, 0:4])
                    nc.sync.dma_start(wq_t[:, 4:8], wq[:, hp, 4:8])
                    qt_sb = hpp.tile([P, TQ], fp16, tag="qt", name="qt_sb")
                    kt_sb = hpp.tile([P, TK], fp16, tag="kt", name="kt_sb")
                    state.update(wq_t=wq_t, qt_sb=qt_sb, kt_sb=kt_sb)

                def dma_wk():
                    wk_t = wpool.tile([P, CC, P], fp16, tag="wk", name="wk_t")
                    nc.sync.dma_start(wk_t[:, 0:4], wk[:, hp, 0:4])
                    nc.sync.dma_start(wk_t[:, 4:8], wk[:, hp, 4:8])
                    state.update(wk_t=wk_t)

                def dma_weights():
                    dma_wq()
                    dma_wk()

                def qt_group(t):
                    ps = proj_ps()
                    for c in range(CC):
                        nc.tensor.matmul(
                            ps[:],
                            state["wq_t"][:, c, :],
                            xq_sb[:, c, ts(t, F)],
                            start=(c == 0),
                            stop=(c == CC - 1),
                        )
                    nc.vector.tensor_scalar_add(
                        state["qt_sb"][:, ts(t, F)],
                        ps[:],
                        bq_sb[:, hp : hp + 1],
                    )

                def kt_group(t):
                    ps = proj_ps()
                    for c in range(CC):
                        nc.tensor.matmul(
                            ps[:],
                            state["wk_t"][:, c, :],
                            xk_sb[:, c, ts(t, F)],
                            start=(c == 0),
                            stop=(c == CC - 1),
                        )
                    nc.vector.tensor_scalar_add(
                        state["kt_sb"][:, ts(t, F)],
                        ps[:],
                        bk_sb[:, hp : hp + 1],
                    )

                items = [dma_weights, lambda: qt_group(0)]
                for t in range(TK // F):
                    items.append(lambda t=t: kt_group(t))
                for t in range(1, QT):
                    items.append(lambda t=t: qt_group(t))
                return state, items, (dma_wq, dma_wk, qt_group, kt_group)

            # ---- global attention stream ----------------------------------
            def s_item(hp, t, kc):
                state = states[hp]
                qt_sb, kt_sb = state["qt_sb"], state["kt_sb"]
                st = stp.tile([P, 2, F], f32, tag="st", name="st")
                # head-pair packed: A rows 0-63, B rows 64-127; the two
                # 64-contraction matmuls row-tile onto disjoint row groups
                # and run concurrently (HW-verified ~one 512-col pass total)
                nc.tensor.matmul(
                    st[:, 0, :],
                    kt_sb[0:DH, ts(kc, P)],
                    qt_sb[0:DH, ts(t, F)],
                    start=True,
                    stop=True,
                )
                nc.tensor.matmul(
                    st[:, 1, :],
                    kt_sb[DH:P, ts(kc, P)],
                    qt_sb[DH:P, ts(t, F)],
                    start=True,
                    stop=True,
                )
                at = apool.tile([P, 2, F], fp16, tag="attn", name="at")
                nc.scalar.activation(at[:], st[:], EXP, scale=SCALE)
                state[("at", t, kc)] = at

            def av_item(hp, t, kc):
                state = states[hp]
                if kc == 0:
                    state[("avA", t)] = avp.tile([P, F], f32, tag="av", name="avA")
                    state[("avB", t)] = avp.tile([P, F], f32, tag="av", name="avB")
                at = state.pop(("at", t, kc))
                avA, avB = state[("avA", t)], state[("avB", t)]
                nc.tensor.matmul(
                    avA[0:65, :],
                    vp_sbs[kc][:, hp, 0:65],
                    at[:, 0, :],
                    start=(kc == 0),
                    stop=(kc == KC - 1),
                )
                nc.tensor.matmul(
                    avB[0:65, :],
                    vp_sbs[kc][:, hp, 65:130],
                    at[:, 1, :],
                    start=(kc == 0),
                    stop=(kc == KC - 1),
                )

            def norm_pre(hp, t):
                # both av psums carry their denominator at row 64; approx
                # reciprocal (18 bits, ample for a softmax denom) on DVE,
                # fp16 convert, then gpsimd broadcast across the head's 64
                # partitions. Emission order lets head B's recip/convert
                # overlap head A's broadcast on the Pool engine.
                state = states[hp]
                bcs = []
                for h in range(2):
                    av = state[("avA", t)] if h == 0 else state[("avB", t)]
                    # shift the denominator row 64 -> 0 with a plain copy
                    # (32-aligned partition shift, HW-proven); the custom
                    # reciprocal op cannot shift lanes and must run at
                    # base partition 0 (base-64 in/out corrupts)
                    dn = npool.tile([P, F], f32, tag="dn", name="dn")
                    nc.vector.tensor_copy(dn[0:1, :], av[64:65, :])
                    rdf = npool.tile([P, F], f32, tag="rdf", name="rdf")
                    nc.vector.reciprocal_approx_fast(rdf[0:1, :], dn[0:1, :])
                    rd = npool.tile([P, F], fp16, tag="rd", name="rd")
                    nc.vector.tensor_copy(rd[0:1, :], rdf[0:1, :])
                    bc = npool.tile([P, F], fp16, tag="bc", name="bc")
                    nc.gpsimd.partition_broadcast(bc[0:DH, :], rd[0:1, :])
                    bcs.append(bc)
                state[("bc", t)] = tuple(bcs)

            def norm_mul(hp, t, h):
                state = states[hp]
                av = state[("avA", t)] if h == 0 else state[("avB", t)]
                bc = state[("bc", t)][h]
                rows = slice(0, DH) if h == 0 else slice(DH, P)
                nc.vector.tensor_mul(
                    ot_sb[rows, hp, ts(t, F)], av[0:DH, :], bc[0:DH, :]
                )

            av_slot = _av_schedule()

            def stream_slots():
                """One closure per S slot; each emits its S round, any AV
                rounds scheduled for the slot, and normalize items due."""
                by_slot = {}
                for a, s in enumerate(av_slot):
                    by_slot.setdefault(s, []).append(a)

                def make(g):
                    def run():
                        hp_s, t_s, kc_s = g // 64, (g // KC) % QT, g % KC
                        s_item(hp_s, t_s, kc_s)
                        for a in by_slot.get(g, ()):
                            hp_a, t_a, kc_a = a // 64, (a // KC) % QT, a % KC
                            av_item(hp_a, t_a, kc_a)
                            if kc_a == KC - 1:
                                norm_pre(hp_a, t_a)
                                norm_mul(hp_a, t_a, 0)
                                norm_mul(hp_a, t_a, 1)
                    return run

                slots = [make(g) for g in range(NROUND)]
                tail = []

                def make_tail(a):
                    def run():
                        hp_a, t_a, kc_a = a // 64, (a // KC) % QT, a % KC
                        av_item(hp_a, t_a, kc_a)
                        if kc_a == KC - 1:
                            norm_pre(hp_a, t_a)
                            norm_mul(hp_a, t_a, 0)
                            norm_mul(hp_a, t_a, 1)
                    return run

                for a, s in enumerate(av_slot):
                    if s >= NROUND:
                        tail.append(make_tail(a))
                return slots, tail

            def y_items():
                def y_group_t(dc, t):
                    ysb = npool.tile([P, F], f32, tag="y", name="ysb", bufs=2)
                    ps = proj_ps()
                    for s in range(HP):
                        nc.tensor.matmul(
                            ps[:],
                            wo_sb[:, dc, s, :],
                            ot_sb[:, s, ts(t, F)],
                            start=(s == 0),
                            stop=(s == HP - 1),
                        )
                    if t == QT - 1:
                        # final q-tile: halve the evict+store so the last
                        # write-out pipelines instead of trailing whole, and
                        # issue the stores from the Act/Pool sequencers —
                        # both idle by now — so descriptor generation does
                        # not serialize on SP behind the last matmuls
                        engines = [nc.scalar, nc.gpsimd]
                        for g in range(2):
                            h = ds(g * (F // 2), F // 2)
                            nc.vector.tensor_copy(ysb[:, h], ps[:, h])
                            engines[(dc + g) % 2].dma_start(
                                yT[:, dc, ds(t * F + g * (F // 2), F // 2)], ysb[:, h]
                            )
                    else:
                        nc.vector.tensor_copy(ysb[:], ps[:])
                        nc.sync.dma_start(yT[:, dc, ts(t, F)], ysb[:])

                # emission-order safety: a read emitted before its writer
                # gets no RAW edge, so a Y group may only be emitted after
                # ALL normalize items writing the ot slices it reads.
                return [
                    [lambda dc=dc, t=t: y_group_t(dc, t) for dc in range(CC)]
                    for t in range(QT)
                ]

            pending = []
            for _rep in range(repeat):
                slots, tail = stream_slots()
                for hp in range(HP):
                    state, pitems, raw = proj_items(hp)
                    if _rep == 0 and hp == 0:
                        # hand-rolled first phase: weights + x DMAs, then
                        # Q/K groups and V' groups threaded through the t0
                        # slots they feed — kt(g) lands just before its
                        # first S consumer and V'(kc) at slot kc, L0 slots
                        # ahead of its AV consumer, so the exp pipeline
                        # starts as early as possible.
                        dma_wq, dma_wk, qt_g, kt_g = raw
                        dma_wq()
                        load_x_head()
                        dma_wk()
                        load_x()
                        qt_g(0)
                        kt_g(0)
                        for kc in range(KC):
                            if kc % 4 == 0 and kc > 0:
                                kt_g(kc // 4)
                            vp_group(kc)
                            slots[kc]()
                        for t in range(1, QT):
                            qt_g(t)
                        pending = slots[KC:64]
                        continue
                    if hp == 0:
                        # rep > 0: the V' projection is part of every body;
                        # thread its 16 groups into the hp0 proj phase
                        # (their first AV consumers run L0+ slots later)
                        pitems = (
                            pitems[:6]
                            + [lambda kc=kc: vp_group(kc) for kc in range(KC)]
                            + pitems[6:]
                        )
                    _interleave(pending, pitems)
                    pending = slots[hp * 64 : (hp + 1) * 64]
                # Y groups for q-tile t interleave with the hp3 slots after
                # hp3's normalize(t) (whose emission slot follows from the
                # AV schedule); the final q-tile's groups and the AV tail
                # carry into the next repeat's projection phase.
                y_gs = y_items()
                hp3 = pending
                cuts = [
                    min(av_slot[3 * 64 + KC * t + KC - 1] - 3 * 64 + 1, 64)
                    for t in range(QT)
                ]
                _interleave(hp3[: cuts[0]], [])
                seg_prev = cuts[0]
                for t in range(QT - 1):
                    seg_end = cuts[t + 1] if t < QT - 2 else 64
                    _interleave(hp3[seg_prev:seg_end], y_gs[t])
                    seg_prev = seg_end
                pending = tail + y_gs[QT - 1]
            for it in pending:
                it()

    nc.compile()
    return nc


_NC_CACHE = None


def _get_nc():
    global _NC_CACHE
    if _NC_CACHE is None:
        _NC_CACHE = build_nc()
    return _NC_CACHE


def _chunk_T(a):
    # [rows, D] f32 -> [P, D//P, rows] fp16 (feature-major chunks)
    return np.ascontiguousarray(
        a.T.reshape(CC, P, a.shape[0]).transpose(1, 0, 2).astype(np.float16)
    )


def _prep_in_maps(query, key, value, Wq, bq, Wk, bk, Wv, bv, Wo, bo):
    HD = HP * P  # 512 output dims per core

    def _shard_w_out(W, h):
        # rows [h*512, (h+1)*512) of a [D, D] torch-Linear weight, laid out
        # [p, hp, c, j] = W[h*512 + hp*128 + p, 128c + j]
        Wh = W[h * HD : (h + 1) * HD, :]
        return np.ascontiguousarray(
            Wh.reshape(HP, P, CC, P).transpose(3, 0, 2, 1).astype(np.float16)
        )

    wv_d, wo_d, bq_d, bk_d = [], [], [], []
    for h in range(2):
        Wvh = Wv[h * HD : (h + 1) * HD, :]
        # wv: [p, c, j] = Wvh[j, 128c + p] (dout-major for 512-wide groups)
        wv_d.append(
            np.ascontiguousarray(
                Wvh.reshape(HD, CC, P).transpose(2, 1, 0).astype(np.float16)
            )
        )
        # wo: [p2, dc, hp, j] = Wo[dc*128+j, h*512 + hp*128 + p2], p2 = a*64+dh
        Woh = Wo[:, h * HD : (h + 1) * HD]
        wo_d.append(
            np.ascontiguousarray(
                Woh.reshape(CC, P, HP, 2, DH)
                .transpose(3, 4, 0, 2, 1)
                .reshape(P, CC, HP, P)
                .astype(np.float16)
            )
        )
        bq_d.append(
            np.ascontiguousarray(
                bq[h * HD : (h + 1) * HD].reshape(HP, P).T.astype(np.float32)
            )
        )
        bk_d.append(
            np.ascontiguousarray(
                bk[h * HD : (h + 1) * HD].reshape(HP, P).T.astype(np.float32)
            )
        )

    xq_b = [_chunk_T(query[b]) for b in range(B)]
    xk_b = [_chunk_T(key[b]) for b in range(B)]
    xv_b = [_chunk_T(value[b]) for b in range(B)]

    in_maps = []
    for c in range(N_CORES):
        b, h = divmod(c, 2)
        in_maps.append(
            {
                "xq": xq_b[b],
                "xk": xk_b[b],
                "xv": xv_b[b],
                "wq": _shard_w_out(Wq, h),
                "wk": _shard_w_out(Wk, h),
                "wv": wv_d[h],
                "wo": wo_d[h],
                "bq": bq_d[h],
                "bk": bk_d[h],
            }
        )
    return in_maps


def _gather(results, bv, bo, Wo):
    crow = (bv.astype(np.float32) @ Wo.T.astype(np.float32) + bo).astype(np.float32)
    out = np.empty((B, T, D), np.float32)
    for b in range(B):
        # two head-half partials per batch; sum them
        y0 = results[2 * b]["yT"]  # [P, CC, TQ] f32 = Y^T partial chunks
        y1 = results[2 * b + 1]["yT"]
        y = (y0 + y1).transpose(1, 0, 2).reshape(D, TQ).T
        out[b] = y + crow
    return out


def _run(inputs, trace=False, **kwargs):
    inputs = {k: np.asarray(v) for k, v in inputs.items()}
    nc = _get_nc()
    in_maps = _prep_in_maps(**inputs)
    res = bass_utils.run_bass_kernel_spmd(
        nc, in_maps, core_ids=list(range(N_CORES)), trace=trace, **kwargs
    )
    out = _gather(res.results, inputs["bv"], inputs["bo"], inputs["Wo"])
    return out, res


def kernel(**inputs):
    out, _ = _run(inputs, trace=False)
    return out


# revision 25
# speedup vs baseline: 1.0542x; 1.0009x over previous
"""Trainium2 Bass kernel for MemoryOptimizedAttention (MHA with projections).

Problem (hardcoded): B=4, T=2048, D=1024, H=16, DH=64, fp32 I/O.

Sharding: 8 cores = (batch b, head-half h) pairs — data parallel on B,
tensor parallel on heads. Each core projects only its 8 heads' Q/K/V
features (columns of Wq/Wk/Wv) for the full 2048-token context — no
cross-core duplication — runs attention for those heads over all 2048
queries, and applies its half of Wo (rows), producing a partial output.
The host gather adds the two partials per batch. No collectives.

Device dataflow (feature-major / transposed layouts throughout):
  QT = Wq_h @ xqT (+bq_h)        [512, 2048]    per head-pair (hp) chunks
  KT = Wk_h @ xkT (+bk_h)        [512, 2048]
  V' = xvT.T @ Wv_hT             [2048, 130/hp] k-major, ones cols at 64/129
  S^T[k,q] = KT_p.T-slice @ QT_p [128, 2, 512]  row-tiled pair (concurrent)
  O'^T = V'^T @ attn^T           [65, 512] per head (denom rides at row 64)
  O^T[p,q] = O' * bcast(1/denom) [128, 4, 2048]  A rows 0-63, B rows 64-127
  Y^T_partial = Wo_h @ O^T       [1024, 2048]   128-deep contraction chunks
bv and bo are folded on the host into a constant row added to the output.

Scheduling: the S->exp stream runs CONTINUOUSLY at one (t,kc) round per
slot across q-tile, head-pair, and repeat-boundaries, so the ScalarE exp
pipe never starves. AV consumption lags the S stream by L0 slots in steady
state; at each q-tile boundary the lag stretches to LB slots (catching up
two AVs per slot afterwards) so the avp PSUM-bank WAR against the previous
tile's normalize chain gets a ~LB-slot window. The normalize chain uses
reciprocal_approx_fast (~0.7us vs 4us for full reciprocal; 18 bits is
plenty for a softmax denominator) and is emission-ordered so head B's
reciprocal overlaps head A's gpsimd broadcast. Projections for the next
head pair interleave into the current pair's attention slots; the final
q-tile's output-projection groups carry into the next repeat.
"""

import sys

for _p in ("/opt/trn_rl_repo",):
    if _p not in sys.path:
        sys.path.insert(0, _p)

import numpy as np

import concourse.bass as bass
import concourse.mybir as mybir
import concourse.tile as tile
from concourse import bacc
from concourse import bass_utils
from concourse.bass import ts, ds

B, T, D, H = 4, 2048, 1024, 16
DH = D // H
SCALE = 1.0 / float(np.sqrt(DH))

P = 128
HP = 4     # head pairs per core (8 heads)
CC = 8     # 128-wide chunks of D (projection contraction)
KC = 16    # 128-wide chunks of the key/context dim (2048)
QT = 4     # 512-wide q tiles per core (full T)
F = 512
TQ = 2048  # q rows per core
TK = 2048  # context rows per core

L0 = 4     # steady AV lag behind the S stream (slots)
LB = 8     # stretched lag at q-tile boundaries (avp WAR window)

fp16 = mybir.dt.float16
f32 = mybir.dt.float32
EXP = mybir.ActivationFunctionType.Exp

N_CORES = 8
NROUND = HP * QT * KC  # 256 attention rounds per body


def _interleave(a_items, b_items):
    """Emit two work-item lists interleaved evenly."""
    na, nb = len(a_items), len(b_items)
    ia = ib = 0
    while ia < na or ib < nb:
        if ia >= na:
            b_items[ib]()
            ib += 1
        elif ib >= nb:
            a_items[ia]()
            ia += 1
        elif ib * na <= ia * nb:
            b_items[ib]()
            ib += 1
        else:
            a_items[ia]()
            ia += 1


def _av_schedule():
    """slot index for each AV round: lag L0 behind its S slot, stretched to
    LB at q-tile boundaries, monotone, at most 2 AV emissions per slot."""
    slots = []
    last = -1
    per_slot = {}
    for a in range(NROUND):
        need = LB if (a % KC == 0 and a > 0) else L0
        s = max(last, a + need)
        while per_slot.get(s, 0) >= 2:
            s += 1
        per_slot[s] = per_slot.get(s, 0) + 1
        slots.append(s)
        last = s
    return slots


def build_nc(repeat=1):
    nc = bacc.Bacc(None, target_bir_lowering=False, debug=False)

    xq = nc.dram_tensor("xq", [P, CC, TQ], fp16, kind="ExternalInput")
    xk = nc.dram_tensor("xk", [P, CC, TK], fp16, kind="ExternalInput")
    xv = nc.dram_tensor("xv", [P, CC, TK], fp16, kind="ExternalInput")
    wq = nc.dram_tensor("wq", [P, HP, CC, P], fp16, kind="ExternalInput")
    wk = nc.dram_tensor("wk", [P, HP, CC, P], fp16, kind="ExternalInput")
    wv = nc.dram_tensor("wv", [P, CC, HP * P], fp16, kind="ExternalInput")
    wo = nc.dram_tensor("wo", [P, CC, HP, P], fp16, kind="ExternalInput")
    bq = nc.dram_tensor("bq", [P, HP], f32, kind="ExternalInput")
    bk = nc.dram_tensor("bk", [P, HP], f32, kind="ExternalInput")
    yT = nc.dram_tensor("yT", [P, CC, TQ], f32, kind="ExternalOutput")

    with tile.TileContext(nc) as tc:
        with (
            tc.tile_pool(name="res", bufs=1) as res,
            tc.tile_pool(name="wpool", bufs=2) as wpool,
            tc.tile_pool(name="hpp", bufs=2) as hpp,
            tc.tile_pool(name="apool", bufs=LB + 3) as apool,
            tc.tile_pool(name="npool", bufs=2) as npool,
            tc.tile_pool(name="mmp", bufs=1, space="PSUM") as mmp,
            tc.tile_pool(name="stp", bufs=2, space="PSUM") as stp,
            tc.tile_pool(name="avp", bufs=2, space="PSUM") as avp,
            tc.tile_pool(name="bcp", bufs=1, space="PSUM") as bcp,
        ):
            bq_sb = res.tile([P, HP], f32)
            bk_sb = res.tile([P, HP], f32)
            # resident x tiles; DMAs are emitted after the first head-pair's
            # weight loads (see below) so the first projections start early
            xq_sb = res.tile([P, CC, TQ], fp16)
            xk_sb = res.tile([P, CC, TK], fp16)
            xv_sb = res.tile([P, CC, TK], fp16)
            wv_sb = res.tile([P, CC, HP * P], fp16)
            wo_sb = res.tile([P, CC, HP, P], fp16)
            # V' for this core's head pairs, k-major, with ones cols at
            # 64/129; one tile per k-chunk keeps access patterns simple for
            # the dependency tracker
            vp_sbs = [
                res.tile([P, HP, 130], fp16, name=f"vp_sb{k}") for k in range(KC)
            ]

            def load_x_head():
                # the first Q-projection matmuls gate the whole dispatch, so
                # their xq chunks go right behind wq at the head of the SP
                # queue, finest-grained first
                nc.sync.dma_start(xq_sb[:, 0, ts(0, F)], xq[:, 0, ts(0, F)])
                nc.sync.dma_start(xq_sb[:, 1, ts(0, F)], xq[:, 1, ts(0, F)])
                for g in range(1, 4):
                    nc.sync.dma_start(
                        xq_sb[:, 2 * g : 2 * g + 2, ts(0, F)],
                        xq[:, 2 * g : 2 * g + 2, ts(0, F)],
                    )

            def load_x():
                # slab DMAs aligned with first consumers, issued from three
                # sequencers in parallel so descriptor generation does not
                # serialize the startup: SP feeds Q, Act feeds K and wv,
                # Pool (SWDGE) feeds the early xv k-chunks
                nc.sync.dma_start(bq_sb[:], bq[:])
                nc.sync.dma_start(bk_sb[:], bk[:])
                nc.scalar.dma_start(xk_sb[:, 0:4, ts(0, F)], xk[:, 0:4, ts(0, F)])
                nc.scalar.dma_start(xk_sb[:, 4:8, ts(0, F)], xk[:, 4:8, ts(0, F)])
                for t in range(1, TK // F):
                    nc.scalar.dma_start(xk_sb[:, :, ts(t, F)], xk[:, :, ts(t, F)])
                nc.gpsimd.dma_start(wv_sb[:], wv[:])
                for kc in range(KC // 2):
                    nc.gpsimd.dma_start(xv_sb[:, :, ts(kc, P)], xv[:, :, ts(kc, P)])
                for t in range(1, QT):
                    nc.sync.dma_start(xq_sb[:, :, ts(t, F)], xq[:, :, ts(t, F)])
                for kc in range(KC // 2, KC):
                    nc.sync.dma_start(xv_sb[:, :, ts(kc, P)], xv[:, :, ts(kc, P)])
                for dc in range(CC):
                    nc.sync.dma_start(wo_sb[:, dc], wo[:, dc])
                for k in range(KC):
                    nc.vector.memset(vp_sbs[k][:, :, 64:65], 1.0)
                    nc.vector.memset(vp_sbs[k][:, :, 129:130], 1.0)

            # O^T, head-pair packed: A dims at partitions 0-63, B at 64-127
            ot_sb = res.tile([P, HP, TQ], fp16)

            # proj-group PSUM ping-pong across mmp/bcp to dodge the
            # single-buffer eviction WAR; positional counter spans all
            # projection/output groups
            pp = [0]

            def proj_ps():
                pool = (mmp, bcp)[pp[0] % 2]
                pp[0] += 1
                return pool.tile(
                    [P, F], f32, tag="proj" if pool is mmp else "bps", name="ps"
                )

            def vp_group(kc):
                # V' projection for all 4 head pairs at once (512 out dims)
                ps = proj_ps()
                for c in range(CC):
                    nc.tensor.matmul(
                        ps[:],
                        xv_sb[:, c, ts(kc, P)],
                        wv_sb[:, c, :],
                        start=(c == 0),
                        stop=(c == CC - 1),
                    )
                ps4 = ps[:].rearrange("p (hp g j) -> p hp g j", hp=HP, g=2)
                vp4 = vp_sbs[kc][:].rearrange("p hp (g x) -> p hp g x", g=2)
                nc.vector.tensor_copy(vp4[:, :, :, 0:64], ps4[:])

            states = {}

            def proj_items(hp):
                state = states.setdefault(hp, {})

                def dma_wq():
                    # halved weight DMAs: the first projection matmul only
                    # needs chunk 0, so it starts as soon as the first half
                    # lands
                    wq_t = wpool.tile([P, CC, P], fp16, tag="wq", name="wq_t")
                    nc.sync.dma_start(wq_t[:, 0:4], wq[:, hp, 0:4])
                    nc.sync.dma_start(wq_t[:, 4:8], wq[:, hp, 4:8])
                    qt_sb = hpp.tile([P, TQ], fp16, tag="qt", name="qt_sb")
                    kt_sb = hpp.tile([P, TK], fp16, tag="kt", name="kt_sb")
                    state.update(wq_t=wq_t, qt_sb=qt_sb, kt_sb=kt_sb)

                def dma_wk():
                    wk_t = wpool.tile([P, CC, P], fp16, tag="wk", name="wk_t")
                    nc.sync.dma_start(wk_t[:, 0:4], wk[:, hp, 0:4])
                    nc.sync.dma_start(wk_t[:, 4:8], wk[:, hp, 4:8])
                    state.update(wk_t=wk_t)

                def dma_weights():
                    dma_wq()
                    dma_wk()

                def qt_group(t):
                    ps = proj_ps()
                    for c in range(CC):
                        nc.tensor.matmul(
                            ps[:],
                            state["wq_t"][:, c, :],
                            xq_sb[:, c, ts(t, F)],
                            start=(c == 0),
                            stop=(c == CC - 1),
                        )
                    nc.vector.tensor_scalar_add(
                        state["qt_sb"][:, ts(t, F)],
                        ps[:],
                        bq_sb[:, hp : hp + 1],
                    )

                def kt_group(t):
                    ps = proj_ps()
                    for c in range(CC):
                        nc.tensor.matmul(
                            ps[:],
                            state["wk_t"][:, c, :],
                            xk_sb[:, c, ts(t, F)],
                            start=(c == 0),
                            stop=(c == CC - 1),
                        )
                    nc.vector.tensor_scalar_add(
                        state["kt_sb"][:, ts(t, F)],
                        ps[:],
                        bk_sb[:, hp : hp + 1],
                    )

                items = [dma_weights, lambda: qt_group(0)]
                for t in range(TK // F):
                    items.append(lambda t=t: kt_group(t))
                for t in range(1, QT):
                    items.append(lambda t=t: qt_group(t))
                return state, items, (dma_wq, dma_wk, qt_group, kt_group)

            # ---- global attention stream ----------------------------------
            def s_item(hp, t, kc):
                state = states[hp]
                qt_sb, kt_sb = state["qt_sb"], state["kt_sb"]
                st = stp.tile([P, 2, F], f32, tag="st", name="st")
                # head-pair packed: A rows 0-63, B rows 64-127; the two
                # 64-contraction matmuls row-tile onto disjoint row groups
                # and run concurrently (HW-verified ~one 512-col pass total)
                nc.tensor.matmul(
                    st[:, 0, :],
                    kt_sb[0:DH, ts(kc, P)],
                    qt_sb[0:DH, ts(t, F)],
                    start=True,
                    stop=True,
                )
                nc.tensor.matmul(
                    st[:, 1, :],
                    kt_sb[DH:P, ts(kc, P)],
                    qt_sb[DH:P, ts(t, F)],
                    start=True,
                    stop=True,
                )
                at = apool.tile([P, 2, F], fp16, tag="attn", name="at")
                nc.scalar.activation(at[:], st[:], EXP, scale=SCALE)
                state[("at", t, kc)] = at

            def av_item(hp, t, kc):
                state = states[hp]
                if kc == 0:
                    state[("avA", t)] = avp.tile([P, F], f32, tag="av", name="avA")
                    state[("avB", t)] = avp.tile([P, F], f32, tag="av", name="avB")
                at = state.pop(("at", t, kc))
                avA, avB = state[("avA", t)], state[("avB", t)]
                nc.tensor.matmul(
                    avA[0:65, :],
                    vp_sbs[kc][:, hp, 0:65],
                    at[:, 0, :],
                    start=(kc == 0),
                    stop=(kc == KC - 1),
                )
                nc.tensor.matmul(
                    avB[0:65, :],
                    vp_sbs[kc][:, hp, 65:130],
                    at[:, 1, :],
                    start=(kc == 0),
                    stop=(kc == KC - 1),
                )

            def norm_pre(hp, t):
                # both av psums carry their denominator at row 64; approx
                # reciprocal (18 bits, ample for a softmax denom) on DVE,
                # fp16 convert, then gpsimd broadcast across the head's 64
                # partitions. Emission order lets head B's recip/convert
                # overlap head A's broadcast on the Pool engine.
                state = states[hp]
                bcs = []
                for h in range(2):
                    av = state[("avA", t)] if h == 0 else state[("avB", t)]
                    # shift the denominator row 64 -> 0 with a plain copy
                    # (32-aligned partition shift, HW-proven), broadcast the
                    # RAW fp32 denominator on the Pool engine, then take the
                    # approx reciprocal over all 64 rows at once — its cost
                    # is partition-count independent, and it must run at
                    # base partition 0 (base-64 in/out corrupts)
                    dn = npool.tile([P, F], f32, tag="dn", name="dn")
                    nc.vector.tensor_copy(dn[0:1, :], av[64:65, :])
                    db = npool.tile([P, F], f32, tag="db", name="db")
                    nc.gpsimd.partition_broadcast(db[0:DH, :], dn[0:1, :])
                    bc = npool.tile([P, F], f32, tag="bc", name="bc")
                    nc.vector.reciprocal_approx_fast(bc[0:DH, :], db[0:DH, :])
                    bcs.append(bc)
                state[("bc", t)] = tuple(bcs)

            def norm_mul(hp, t, h):
                state = states[hp]
                av = state[("avA", t)] if h == 0 else state[("avB", t)]
                bc = state[("bc", t)][h]
                rows = slice(0, DH) if h == 0 else slice(DH, P)
                nc.vector.tensor_mul(
                    ot_sb[rows, hp, ts(t, F)], av[0:DH, :], bc[0:DH, :]
                )

            av_slot = _av_schedule()

            def stream_slots():
                """One closure per S slot; each emits its S round, any AV
                rounds scheduled for the slot, and normalize items due."""
                by_slot = {}
                for a, s in enumerate(av_slot):
                    by_slot.setdefault(s, []).append(a)

                def make(g):
                    def run():
                        hp_s, t_s, kc_s = g // 64, (g // KC) % QT, g % KC
                        s_item(hp_s, t_s, kc_s)
                        for a in by_slot.get(g, ()):
                            hp_a, t_a, kc_a = a // 64, (a // KC) % QT, a % KC
                            av_item(hp_a, t_a, kc_a)
                            if kc_a == KC - 1:
                                norm_pre(hp_a, t_a)
                                norm_mul(hp_a, t_a, 0)
                                norm_mul(hp_a, t_a, 1)
                    return run

                slots = [make(g) for g in range(NROUND)]
                tail = []

                def make_tail(a):
                    def run():
                        hp_a, t_a, kc_a = a // 64, (a // KC) % QT, a % KC
                        av_item(hp_a, t_a, kc_a)
                        if kc_a == KC - 1:
                            norm_pre(hp_a, t_a)
                            norm_mul(hp_a, t_a, 0)
                            norm_mul(hp_a, t_a, 1)
                    return run

                for a, s in enumerate(av_slot):
                    if s >= NROUND:
                        tail.append(make_tail(a))
                return slots, tail

            def y_items():
                def y_group_t(dc, t):
                    ysb = npool.tile([P, F], f32, tag="y", name="ysb", bufs=2)
                    ps = proj_ps()
                    for s in range(HP):
                        nc.tensor.matmul(
                            ps[:],
                            wo_sb[:, dc, s, :],
                            ot_sb[:, s, ts(t, F)],
                            start=(s == 0),
                            stop=(s == HP - 1),
                        )
                    if t == QT - 1:
                        # final q-tile: halve the evict+store so the last
                        # write-out pipelines instead of trailing whole, and
                        # issue the stores from the Act/Pool sequencers —
                        # both idle by now — so descriptor generation does
                        # not serialize on SP behind the last matmuls
                        engines = [nc.scalar, nc.gpsimd]
                        for g in range(2):
                            h = ds(g * (F // 2), F // 2)
                            nc.vector.tensor_copy(ysb[:, h], ps[:, h])
                            engines[(dc + g) % 2].dma_start(
                                yT[:, dc, ds(t * F + g * (F // 2), F // 2)], ysb[:, h]
                            )
                    else:
                        nc.vector.tensor_copy(ysb[:], ps[:])
                        nc.sync.dma_start(yT[:, dc, ts(t, F)], ysb[:])

                # emission-order safety: a read emitted before its writer
                # gets no RAW edge, so a Y group may only be emitted after
                # ALL normalize items writing the ot slices it reads.
                return [
                    [lambda dc=dc, t=t: y_group_t(dc, t) for dc in range(CC)]
                    for t in range(QT)
                ]

            pending = []
            for _rep in range(repeat):
                slots, tail = stream_slots()
                for hp in range(HP):
                    state, pitems, raw = proj_items(hp)
                    if _rep == 0 and hp == 0:
                        # hand-rolled first phase: weights + x DMAs, then
                        # Q/K groups and V' groups threaded through the t0
                        # slots they feed — kt(g) lands just before its
                        # first S consumer and V'(kc) at slot kc, L0 slots
                        # ahead of its AV consumer, so the exp pipeline
                        # starts as early as possible.
                        dma_wq, dma_wk, qt_g, kt_g = raw
                        dma_wq()
                        load_x_head()
                        dma_wk()
                        load_x()
                        qt_g(0)
                        kt_g(0)
                        for kc in range(KC):
                            if kc % 4 == 0 and kc > 0:
                                kt_g(kc // 4)
                            vp_group(kc)
                            slots[kc]()
                        for t in range(1, QT):
                            qt_g(t)
                        pending = slots[KC:64]
                        continue
                    if hp == 0:
                        # rep > 0: the V' projection is part of every body;
                        # thread its 16 groups into the hp0 proj phase
                        # (their first AV consumers run L0+ slots later)
                        pitems = (
                            pitems[:6]
                            + [lambda kc=kc: vp_group(kc) for kc in range(KC)]
                            + pitems[6:]
                        )
                    _interleave(pending, pitems)
                    pending = slots[hp * 64 : (hp + 1) * 64]
                # Y groups for q-tile t interleave with the hp3 slots after
                # hp3's normalize(t) (whose emission slot follows from the
                # AV schedule); the final q-tile's groups and the AV tail
                # carry into the next repeat's projection phase.
                y_gs = y_items()
                hp3 = pending
                cuts = [
                    min(av_slot[3 * 64 + KC * t + KC - 1] - 3 * 64 + 1, 64)
                    for t in range(QT)
                ]
                _interleave(hp3[: cuts[0]], [])
                seg_prev = cuts[0]
                for t in range(QT - 1):
                    seg_end = cuts[t + 1] if t < QT - 2 else 64
                    _interleave(hp3[seg_prev:seg_end], y_gs[t])
                    seg_prev = seg_end
                pending = tail + y_gs[QT - 1]
            for it in pending:
                it()

    nc.compile()
    return nc


_NC_CACHE = None


def _get_nc():
    global _NC_CACHE
    if _NC_CACHE is None:
        _NC_CACHE = build_nc()
    return _NC_CACHE


def _chunk_T(a):
    # [rows, D] f32 -> [P, D//P, rows] fp16 (feature-major chunks)
    return np.ascontiguousarray(
        a.T.reshape(CC, P, a.shape[0]).transpose(1, 0, 2).astype(np.float16)
    )


def _prep_in_maps(query, key, value, Wq, bq, Wk, bk, Wv, bv, Wo, bo):
    HD = HP * P  # 512 output dims per core

    def _shard_w_out(W, h):
        # rows [h*512, (h+1)*512) of a [D, D] torch-Linear weight, laid out
        # [p, hp, c, j] = W[h*512 + hp*128 + p, 128c + j]
        Wh = W[h * HD : (h + 1) * HD, :]
        return np.ascontiguousarray(
            Wh.reshape(HP, P, CC, P).transpose(3, 0, 2, 1).astype(np.float16)
        )

    wv_d, wo_d, bq_d, bk_d = [], [], [], []
    for h in range(2):
        Wvh = Wv[h * HD : (h + 1) * HD, :]
        # wv: [p, c, j] = Wvh[j, 128c + p] (dout-major for 512-wide groups)
        wv_d.append(
            np.ascontiguousarray(
                Wvh.reshape(HD, CC, P).transpose(2, 1, 0).astype(np.float16)
            )
        )
        # wo: [p2, dc, hp, j] = Wo[dc*128+j, h*512 + hp*128 + p2], p2 = a*64+dh
        Woh = Wo[:, h * HD : (h + 1) * HD]
        wo_d.append(
            np.ascontiguousarray(
                Woh.reshape(CC, P, HP, 2, DH)
                .transpose(3, 4, 0, 2, 1)
                .reshape(P, CC, HP, P)
                .astype(np.float16)
            )
        )
        bq_d.append(
            np.ascontiguousarray(
                bq[h * HD : (h + 1) * HD].reshape(HP, P).T.astype(np.float32)
            )
        )
        bk_d.append(
            np.ascontiguousarray(
                bk[h * HD : (h + 1) * HD].reshape(HP, P).T.astype(np.float32)
            )
        )

    xq_b = [_chunk_T(query[b]) for b in range(B)]
    xk_b = [_chunk_T(key[b]) for b in range(B)]
    xv_b = [_chunk_T(value[b]) for b in range(B)]

    in_maps = []
    for c in range(N_CORES):
        b, h = divmod(c, 2)
        in_maps.append(
            {
                "xq": xq_b[b],
                "xk": xk_b[b],
                "xv": xv_b[b],
                "wq": _shard_w_out(Wq, h),
                "wk": _shard_w_out(Wk, h),
                "wv": wv_d[h],
                "wo": wo_d[h],
                "bq": bq_d[h],
                "bk": bk_d[h],
            }
        )
    return in_maps


def _gather(results, bv, bo, Wo):
    crow = (bv.astype(np.float32) @ Wo.T.astype(np.float32) + bo).astype(np.float32)
    out = np.empty((B, T, D), np.float32)
    for b in range(B):
        # two head-half partials per batch; sum them
        y0 = results[2 * b]["yT"]  # [P, CC, TQ] f32 = Y^T partial chunks
        y1 = results[2 * b + 1]["yT"]
        y = (y0 + y1).transpose(1, 0, 2).reshape(D, TQ).T
        out[b] = y + crow
    return out


def _run(inputs, trace=False, **kwargs):
    inputs = {k: np.asarray(v) for k, v in inputs.items()}
    nc = _get_nc()
    in_maps = _prep_in_maps(**inputs)
    res = bass_utils.run_bass_kernel_spmd(
        nc, in_maps, core_ids=list(range(N_CORES)), trace=trace, **kwargs
    )
    out = _gather(res.results, inputs["bv"], inputs["bo"], inputs["Wo"])
    return out, res


def kernel(**inputs):
    out, _ = _run(inputs, trace=False)
    return out


# revision 27
# speedup vs baseline: 1.0554x; 1.0012x over previous
"""Trainium2 Bass kernel for MemoryOptimizedAttention (MHA with projections).

Problem (hardcoded): B=4, T=2048, D=1024, H=16, DH=64, fp32 I/O.

Sharding: 8 cores = (batch b, head-half h) pairs — data parallel on B,
tensor parallel on heads. Each core projects only its 8 heads' Q/K/V
features (columns of Wq/Wk/Wv) for the full 2048-token context — no
cross-core duplication — runs attention for those heads over all 2048
queries, and applies its half of Wo (rows), producing a partial output.
The host gather adds the two partials per batch. No collectives.

Device dataflow (feature-major / transposed layouts throughout):
  QT = Wq_h @ xqT (+bq_h)        [512, 2048]    per head-pair (hp) chunks
  KT = Wk_h @ xkT (+bk_h)        [512, 2048]
  V' = xvT.T @ Wv_hT             [2048, 130/hp] k-major, ones cols at 64/129
  S^T[k,q] = KT_p.T-slice @ QT_p [128, 2, 512]  row-tiled pair (concurrent)
  O'^T = V'^T @ attn^T           [65, 512] per head (denom rides at row 64)
  O^T[p,q] = O' * bcast(1/denom) [128, 4, 2048]  A rows 0-63, B rows 64-127
  Y^T_partial = Wo_h @ O^T       [1024, 2048]   128-deep contraction chunks
bv and bo are folded on the host into a constant row added to the output.

Scheduling: the S->exp stream runs CONTINUOUSLY at one (t,kc) round per
slot across q-tile, head-pair, and repeat-boundaries, so the ScalarE exp
pipe never starves. AV consumption lags the S stream by L0 slots in steady
state; at each q-tile boundary the lag stretches to LB slots (catching up
two AVs per slot afterwards) so the avp PSUM-bank WAR against the previous
tile's normalize chain gets a ~LB-slot window. The normalize chain uses
reciprocal_approx_fast (~0.7us vs 4us for full reciprocal; 18 bits is
plenty for a softmax denominator) and is emission-ordered so head B's
reciprocal overlaps head A's gpsimd broadcast. Projections for the next
head pair interleave into the current pair's attention slots; the final
q-tile's output-projection groups carry into the next repeat.
"""

import sys

for _p in ("/opt/trn_rl_repo",):
    if _p not in sys.path:
        sys.path.insert(0, _p)

import numpy as np

import concourse.bass as bass
import concourse.mybir as mybir
import concourse.tile as tile
from concourse import bacc
from concourse import bass_utils
from concourse.bass import ts, ds

B, T, D, H = 4, 2048, 1024, 16
DH = D // H
SCALE = 1.0 / float(np.sqrt(DH))

P = 128
HP = 4     # head pairs per core (8 heads)
CC = 8     # 128-wide chunks of D (projection contraction)
KC = 16    # 128-wide chunks of the key/context dim (2048)
QT = 4     # 512-wide q tiles per core (full T)
F = 512
TQ = 2048  # q rows per core
TK = 2048  # context rows per core

L0 = 4     # steady AV lag behind the S stream (slots)
LB = 8     # stretched lag at q-tile boundaries (avp WAR window)

fp16 = mybir.dt.float16
f32 = mybir.dt.float32
EXP = mybir.ActivationFunctionType.Exp

N_CORES = 8
NROUND = HP * QT * KC  # 256 attention rounds per body


def _interleave(a_items, b_items):
    """Emit two work-item lists interleaved evenly."""
    na, nb = len(a_items), len(b_items)
    ia = ib = 0
    while ia < na or ib < nb:
        if ia >= na:
            b_items[ib]()
            ib += 1
        elif ib >= nb:
            a_items[ia]()
            ia += 1
        elif ib * na <= ia * nb:
            b_items[ib]()
            ib += 1
        else:
            a_items[ia]()
            ia += 1


def _av_schedule():
    """slot index for each AV round: lag L0 behind its S slot, stretched to
    LB at q-tile boundaries, monotone, at most 2 AV emissions per slot."""
    slots = []
    last = -1
    per_slot = {}
    for a in range(NROUND):
        need = LB if (a % KC == 0 and a > 0) else L0
        s = max(last, a + need)
        while per_slot.get(s, 0) >= 2:
            s += 1
        per_slot[s] = per_slot.get(s, 0) + 1
        slots.append(s)
        last = s
    return slots


def build_nc(repeat=1):
    nc = bacc.Bacc(None, target_bir_lowering=False, debug=False)

    xq = nc.dram_tensor("xq", [P, CC, TQ], fp16, kind="ExternalInput")
    xk = nc.dram_tensor("xk", [P, CC, TK], fp16, kind="ExternalInput")
    xv = nc.dram_tensor("xv", [P, CC, TK], fp16, kind="ExternalInput")
    wq = nc.dram_tensor("wq", [P, HP, CC, P], fp16, kind="ExternalInput")
    wk = nc.dram_tensor("wk", [P, HP, CC, P], fp16, kind="ExternalInput")
    wv = nc.dram_tensor("wv", [P, CC, HP * P], fp16, kind="ExternalInput")
    wo = nc.dram_tensor("wo", [P, CC, HP, P], fp16, kind="ExternalInput")
    bq = nc.dram_tensor("bq", [P, HP], f32, kind="ExternalInput")
    bk = nc.dram_tensor("bk", [P, HP], f32, kind="ExternalInput")
    yT = nc.dram_tensor("yT", [P, CC, TQ], f32, kind="ExternalOutput")

    with tile.TileContext(nc) as tc:
        with (
            tc.tile_pool(name="res", bufs=1) as res,
            tc.tile_pool(name="wpool", bufs=2) as wpool,
            tc.tile_pool(name="hpp", bufs=2) as hpp,
            tc.tile_pool(name="apool", bufs=LB + 3) as apool,
            tc.tile_pool(name="npool", bufs=2) as npool,
            tc.tile_pool(name="mmp", bufs=1, space="PSUM") as mmp,
            tc.tile_pool(name="stp", bufs=2, space="PSUM") as stp,
            tc.tile_pool(name="avp", bufs=2, space="PSUM") as avp,
            tc.tile_pool(name="bcp", bufs=1, space="PSUM") as bcp,
        ):
            bq_sb = res.tile([P, HP], f32)
            bk_sb = res.tile([P, HP], f32)
            # resident x tiles; DMAs are emitted after the first head-pair's
            # weight loads (see below) so the first projections start early
            xq_sb = res.tile([P, CC, TQ], fp16)
            xk_sb = res.tile([P, CC, TK], fp16)
            xv_sb = res.tile([P, CC, TK], fp16)
            wv_sb = res.tile([P, CC, HP * P], fp16)
            wo_sb = res.tile([P, CC, HP, P], fp16)
            # V' for this core's head pairs, k-major, with ones cols at
            # 64/129; one tile per k-chunk keeps access patterns simple for
            # the dependency tracker
            vp_sbs = [
                res.tile([P, HP, 130], fp16, name=f"vp_sb{k}") for k in range(KC)
            ]

            def load_x_head():
                # the first Q-projection matmuls gate the whole dispatch, so
                # their xq chunks go right behind wq at the head of the SP
                # queue, finest-grained first
                nc.sync.dma_start(xq_sb[:, 0, ts(0, F)], xq[:, 0, ts(0, F)])
                nc.sync.dma_start(xq_sb[:, 1, ts(0, F)], xq[:, 1, ts(0, F)])
                for g in range(1, 4):
                    nc.sync.dma_start(
                        xq_sb[:, 2 * g : 2 * g + 2, ts(0, F)],
                        xq[:, 2 * g : 2 * g + 2, ts(0, F)],
                    )

            def load_x():
                # slab DMAs aligned with first consumers, issued from three
                # sequencers in parallel so descriptor generation does not
                # serialize the startup: SP feeds Q, Act feeds K and wv,
                # Pool (SWDGE) feeds the early xv k-chunks
                nc.sync.dma_start(bq_sb[:], bq[:])
                nc.sync.dma_start(bk_sb[:], bk[:])
                nc.scalar.dma_start(xk_sb[:, 0:4, ts(0, F)], xk[:, 0:4, ts(0, F)])
                nc.scalar.dma_start(xk_sb[:, 4:8, ts(0, F)], xk[:, 4:8, ts(0, F)])
                for t in range(1, TK // F):
                    nc.scalar.dma_start(xk_sb[:, :, ts(t, F)], xk[:, :, ts(t, F)])
                nc.gpsimd.dma_start(wv_sb[:], wv[:])
                for kc in range(KC // 2):
                    nc.gpsimd.dma_start(xv_sb[:, :, ts(kc, P)], xv[:, :, ts(kc, P)])
                for t in range(1, QT):
                    nc.sync.dma_start(xq_sb[:, :, ts(t, F)], xq[:, :, ts(t, F)])
                for kc in range(KC // 2, KC):
                    nc.sync.dma_start(xv_sb[:, :, ts(kc, P)], xv[:, :, ts(kc, P)])
                for dc in range(CC):
                    nc.sync.dma_start(wo_sb[:, dc], wo[:, dc])
                for k in range(KC):
                    nc.vector.memset(vp_sbs[k][:, :, 64:65], 1.0)
                    nc.vector.memset(vp_sbs[k][:, :, 129:130], 1.0)

            # O^T, head-pair packed: A dims at partitions 0-63, B at 64-127
            ot_sb = res.tile([P, HP, TQ], fp16)

            # proj-group PSUM ping-pong across mmp/bcp to dodge the
            # single-buffer eviction WAR; positional counter spans all
            # projection/output groups
            pp = [0]

            def proj_ps():
                pool = (mmp, bcp)[pp[0] % 2]
                pp[0] += 1
                return pool.tile(
                    [P, F], f32, tag="proj" if pool is mmp else "bps", name="ps"
                )

            def vp_group(kc):
                # V' projection for all 4 head pairs at once (512 out dims)
                ps = proj_ps()
                for c in range(CC):
                    nc.tensor.matmul(
                        ps[:],
                        xv_sb[:, c, ts(kc, P)],
                        wv_sb[:, c, :],
                        start=(c == 0),
                        stop=(c == CC - 1),
                    )
                ps4 = ps[:].rearrange("p (hp g j) -> p hp g j", hp=HP, g=2)
                vp4 = vp_sbs[kc][:].rearrange("p hp (g x) -> p hp g x", g=2)
                nc.vector.tensor_copy(vp4[:, :, :, 0:64], ps4[:])

            states = {}

            def proj_items(hp):
                state = states.setdefault(hp, {})

                def dma_wq():
                    # halved weight DMAs: the first projection matmul only
                    # needs chunk 0, so it starts as soon as the first half
                    # lands
                    wq_t = wpool.tile([P, CC, P], fp16, tag="wq", name="wq_t")
                    nc.sync.dma_start(wq_t[:, 0:4], wq[:, hp, 0:4])
                    nc.sync.dma_start(wq_t[:, 4:8], wq[:, hp, 4:8])
                    qt_sb = hpp.tile([P, TQ], fp16, tag="qt", name="qt_sb")
                    kt_sb = hpp.tile([P, TK], fp16, tag="kt", name="kt_sb")
                    state.update(wq_t=wq_t, qt_sb=qt_sb, kt_sb=kt_sb)

                def dma_wk():
                    wk_t = wpool.tile([P, CC, P], fp16, tag="wk", name="wk_t")
                    nc.sync.dma_start(wk_t[:, 0:4], wk[:, hp, 0:4])
                    nc.sync.dma_start(wk_t[:, 4:8], wk[:, hp, 4:8])
                    state.update(wk_t=wk_t)

                def dma_weights():
                    dma_wq()
                    dma_wk()

                def qt_group(t):
                    ps = proj_ps()
                    for c in range(CC):
                        nc.tensor.matmul(
                            ps[:],
                            state["wq_t"][:, c, :],
                            xq_sb[:, c, ts(t, F)],
                            start=(c == 0),
                            stop=(c == CC - 1),
                        )
                    nc.vector.tensor_scalar_add(
                        state["qt_sb"][:, ts(t, F)],
                        ps[:],
                        bq_sb[:, hp : hp + 1],
                    )

                def kt_group(t):
                    ps = proj_ps()
                    for c in range(CC):
                        nc.tensor.matmul(
                            ps[:],
                            state["wk_t"][:, c, :],
                            xk_sb[:, c, ts(t, F)],
                            start=(c == 0),
                            stop=(c == CC - 1),
                        )
                    nc.vector.tensor_scalar_add(
                        state["kt_sb"][:, ts(t, F)],
                        ps[:],
                        bk_sb[:, hp : hp + 1],
                    )

                items = [dma_weights, lambda: qt_group(0)]
                for t in range(TK // F):
                    items.append(lambda t=t: kt_group(t))
                for t in range(1, QT):
                    items.append(lambda t=t: qt_group(t))
                return state, items, (dma_wq, dma_wk, qt_group, kt_group)

            # ---- global attention stream ----------------------------------
            def s_item(hp, t, kc):
                state = states[hp]
                qt_sb, kt_sb = state["qt_sb"], state["kt_sb"]
                st = stp.tile([P, 2, F], f32, tag="st", name="st")
                # head-pair packed: A rows 0-63, B rows 64-127; the two
                # 64-contraction matmuls row-tile onto disjoint row groups
                # and run concurrently (HW-verified ~one 512-col pass total)
                nc.tensor.matmul(
                    st[:, 0, :],
                    kt_sb[0:DH, ts(kc, P)],
                    qt_sb[0:DH, ts(t, F)],
                    start=True,
                    stop=True,
                )
                nc.tensor.matmul(
                    st[:, 1, :],
                    kt_sb[DH:P, ts(kc, P)],
                    qt_sb[DH:P, ts(t, F)],
                    start=True,
                    stop=True,
                )
                at = apool.tile([P, 2, F], fp16, tag="attn", name="at")
                nc.scalar.activation(at[:], st[:], EXP, scale=SCALE)
                state[("at", t, kc)] = at

            def av_item(hp, t, kc):
                state = states[hp]
                if kc == 0:
                    state[("avA", t)] = avp.tile([P, F], f32, tag="av", name="avA")
                    state[("avB", t)] = avp.tile([P, F], f32, tag="av", name="avB")
                at = state.pop(("at", t, kc))
                avA, avB = state[("avA", t)], state[("avB", t)]
                nc.tensor.matmul(
                    avA[0:65, :],
                    vp_sbs[kc][:, hp, 0:65],
                    at[:, 0, :],
                    start=(kc == 0),
                    stop=(kc == KC - 1),
                )
                nc.tensor.matmul(
                    avB[0:65, :],
                    vp_sbs[kc][:, hp, 65:130],
                    at[:, 1, :],
                    start=(kc == 0),
                    stop=(kc == KC - 1),
                )

            def norm_pre(hp, t):
                # both av psums carry their denominator at row 64; approx
                # reciprocal (18 bits, ample for a softmax denom) on DVE,
                # fp16 convert, then gpsimd broadcast across the head's 64
                # partitions. Emission order lets head B's recip/convert
                # overlap head A's broadcast on the Pool engine.
                state = states[hp]
                bcs = []
                for h in range(2):
                    av = state[("avA", t)] if h == 0 else state[("avB", t)]
                    # shift the denominator row 64 -> 0 with a plain copy
                    # (32-aligned partition shift, HW-proven), broadcast the
                    # RAW fp32 denominator on the Pool engine, then take the
                    # approx reciprocal over all 64 rows at once — its cost
                    # is partition-count independent, and it must run at
                    # base partition 0 (base-64 in/out corrupts)
                    dn = npool.tile([P, F], f32, tag="dn", name="dn")
                    nc.vector.tensor_copy(dn[0:1, :], av[64:65, :])
                    db = npool.tile([P, F], f32, tag="db", name="db")
                    nc.gpsimd.partition_broadcast(db[0:DH, :], dn[0:1, :])
                    bc = npool.tile([P, F], f32, tag="bc", name="bc")
                    nc.vector.reciprocal_approx_fast(bc[0:DH, :], db[0:DH, :])
                    bcs.append(bc)
                state[("bc", t)] = tuple(bcs)

            def norm_mul(hp, t, h):
                state = states[hp]
                av = state[("avA", t)] if h == 0 else state[("avB", t)]
                bc = state[("bc", t)][h]
                rows = slice(0, DH) if h == 0 else slice(DH, P)
                nc.vector.tensor_mul(
                    ot_sb[rows, hp, ts(t, F)], av[0:DH, :], bc[0:DH, :]
                )

            av_slot = _av_schedule()

            def stream_slots():
                """One closure per S slot; each emits its S round, any AV
                rounds scheduled for the slot, and normalize items due."""
                by_slot = {}
                for a, s in enumerate(av_slot):
                    by_slot.setdefault(s, []).append(a)

                def make(g):
                    def run():
                        hp_s, t_s, kc_s = g // 64, (g // KC) % QT, g % KC
                        s_item(hp_s, t_s, kc_s)
                        for a in by_slot.get(g, ()):
                            hp_a, t_a, kc_a = a // 64, (a // KC) % QT, a % KC
                            av_item(hp_a, t_a, kc_a)
                            if kc_a == KC - 1:
                                norm_pre(hp_a, t_a)
                                norm_mul(hp_a, t_a, 0)
                                norm_mul(hp_a, t_a, 1)
                    return run

                slots = [make(g) for g in range(NROUND)]
                tail = []

                def make_tail(a):
                    def run():
                        hp_a, t_a, kc_a = a // 64, (a // KC) % QT, a % KC
                        av_item(hp_a, t_a, kc_a)
                        if kc_a == KC - 1:
                            norm_pre(hp_a, t_a)
                            norm_mul(hp_a, t_a, 0)
                            norm_mul(hp_a, t_a, 1)
                    return run

                for a, s in enumerate(av_slot):
                    if s >= NROUND:
                        tail.append(make_tail(a))
                return slots, tail

            def y_items():
                def y_group_t(dc, t):
                    ysb = npool.tile([P, F], f32, tag="y", name="ysb", bufs=2)
                    ps = proj_ps()
                    for s in range(HP):
                        nc.tensor.matmul(
                            ps[:],
                            wo_sb[:, dc, s, :],
                            ot_sb[:, s, ts(t, F)],
                            start=(s == 0),
                            stop=(s == HP - 1),
                        )
                    if t == QT - 1:
                        # final q-tile: halve the evict+store so the last
                        # write-out pipelines instead of trailing whole, and
                        # issue the stores from the Act/Pool sequencers —
                        # both idle by now — so descriptor generation does
                        # not serialize on SP behind the last matmuls
                        engines = [nc.scalar, nc.gpsimd]
                        for g in range(2):
                            h = ds(g * (F // 2), F // 2)
                            nc.vector.tensor_copy(ysb[:, h], ps[:, h])
                            engines[(dc + g) % 2].dma_start(
                                yT[:, dc, ds(t * F + g * (F // 2), F // 2)], ysb[:, h]
                            )
                    else:
                        nc.vector.tensor_copy(ysb[:], ps[:])
                        nc.sync.dma_start(yT[:, dc, ts(t, F)], ysb[:])

                # emission-order safety: a read emitted before its writer
                # gets no RAW edge, so a Y group may only be emitted after
                # ALL normalize items writing the ot slices it reads.
                return [
                    [lambda dc=dc, t=t: y_group_t(dc, t) for dc in range(CC)]
                    for t in range(QT)
                ]

            pending = []
            hp0_carried = False  # rep>0: hp0 proj already emitted via lookahead
            for _rep in range(repeat):
                slots, tail = stream_slots()
                for hp in range(HP):
                    if hp == 0 and hp0_carried:
                        # qt/kt/dma of this rep's hp0 were interleaved into
                        # the previous rep's hp3 idle slots; only the V'
                        # groups remain for the boundary phase
                        pitems = [lambda kc=kc: vp_group(kc) for kc in range(KC)]
                        _interleave(pending, pitems)
                        pending = slots[0:64]
                        continue
                    state, pitems, raw = proj_items(hp)
                    if _rep == 0 and hp == 0:
                        # hand-rolled first phase: weights + x DMAs, then
                        # Q/K groups and V' groups threaded through the t0
                        # slots they feed — kt(g) lands just before its
                        # first S consumer and V'(kc) at slot kc, L0 slots
                        # ahead of its AV consumer, so the exp pipeline
                        # starts as early as possible.
                        dma_wq, dma_wk, qt_g, kt_g = raw
                        dma_wq()
                        load_x_head()
                        dma_wk()
                        load_x()
                        qt_g(0)
                        kt_g(0)
                        for kc in range(KC):
                            if kc % 4 == 0 and kc > 0:
                                kt_g(kc // 4)
                            vp_group(kc)
                            slots[kc]()
                        for t in range(1, QT):
                            qt_g(t)
                        pending = slots[KC:64]
                        continue
                    if hp == 0:
                        # rep > 0: the V' projection is part of every body;
                        # thread its 16 groups into the hp0 proj phase
                        # (their first AV consumers run L0+ slots later)
                        pitems = (
                            pitems[:6]
                            + [lambda kc=kc: vp_group(kc) for kc in range(KC)]
                            + pitems[6:]
                        )
                    _interleave(pending, pitems)
                    pending = slots[hp * 64 : (hp + 1) * 64]
                # Y groups for q-tile t interleave with the hp3 slots after
                # hp3's normalize(t) (whose emission slot follows from the
                # AV schedule); the final q-tile's groups and the AV tail
                # carry into the next repeat's projection phase.
                y_gs = y_items()
                hp3 = pending
                cuts = [
                    min(av_slot[3 * 64 + KC * t + KC - 1] - 3 * 64 + 1, 64)
                    for t in range(QT)
                ]
                # lookahead: the next rep's hp0 qt/kt/weight-DMA groups fill
                # hp3's first slots (PE-underloaded until y(t0) unlocks) —
                # their only upstream deps (xq/xk residents, hpp buffer of
                # hp2) are already free by emission here
                if _rep + 1 < repeat:
                    _, next_pitems, _ = proj_items(0)
                    _interleave(hp3[: cuts[0]], next_pitems)
                    hp0_carried = True
                else:
                    _interleave(hp3[: cuts[0]], [])
                seg_prev = cuts[0]
                for t in range(QT - 1):
                    seg_end = cuts[t + 1] if t < QT - 2 else 64
                    _interleave(hp3[seg_prev:seg_end], y_gs[t])
                    seg_prev = seg_end
                pending = tail + y_gs[QT - 1]
            for it in pending:
                it()

    nc.compile()
    return nc


_NC_CACHE = None


def _get_nc():
    global _NC_CACHE
    if _NC_CACHE is None:
        _NC_CACHE = build_nc()
    return _NC_CACHE


def _chunk_T(a):
    # [rows, D] f32 -> [P, D//P, rows] fp16 (feature-major chunks)
    return np.ascontiguousarray(
        a.T.reshape(CC, P, a.shape[0]).transpose(1, 0, 2).astype(np.float16)
    )


def _prep_in_maps(query, key, value, Wq, bq, Wk, bk, Wv, bv, Wo, bo):
    HD = HP * P  # 512 output dims per core

    def _shard_w_out(W, h):
        # rows [h*512, (h+1)*512) of a [D, D] torch-Linear weight, laid out
        # [p, hp, c, j] = W[h*512 + hp*128 + p, 128c + j]
        Wh = W[h * HD : (h + 1) * HD, :]
        return np.ascontiguousarray(
            Wh.reshape(HP, P, CC, P).transpose(3, 0, 2, 1).astype(np.float16)
        )

    wv_d, wo_d, bq_d, bk_d = [], [], [], []
    for h in range(2):
        Wvh = Wv[h * HD : (h + 1) * HD, :]
        # wv: [p, c, j] = Wvh[j, 128c + p] (dout-major for 512-wide groups)
        wv_d.append(
            np.ascontiguousarray(
                Wvh.reshape(HD, CC, P).transpose(2, 1, 0).astype(np.float16)
            )
        )
        # wo: [p2, dc, hp, j] = Wo[dc*128+j, h*512 + hp*128 + p2], p2 = a*64+dh
        Woh = Wo[:, h * HD : (h + 1) * HD]
        wo_d.append(
            np.ascontiguousarray(
                Woh.reshape(CC, P, HP, 2, DH)
                .transpose(3, 4, 0, 2, 1)
                .reshape(P, CC, HP, P)
                .astype(np.float16)
            )
        )
        bq_d.append(
            np.ascontiguousarray(
                bq[h * HD : (h + 1) * HD].reshape(HP, P).T.astype(np.float32)
            )
        )
        bk_d.append(
            np.ascontiguousarray(
                bk[h * HD : (h + 1) * HD].reshape(HP, P).T.astype(np.float32)
            )
        )

    xq_b = [_chunk_T(query[b]) for b in range(B)]
    xk_b = [_chunk_T(key[b]) for b in range(B)]
    xv_b = [_chunk_T(value[b]) for b in range(B)]

    in_maps = []
    for c in range(N_CORES):
        b, h = divmod(c, 2)
        in_maps.append(
            {
                "xq": xq_b[b],
                "xk": xk_b[b],
                "xv": xv_b[b],
                "wq": _shard_w_out(Wq, h),
                "wk": _shard_w_out(Wk, h),
                "wv": wv_d[h],
                "wo": wo_d[h],
                "bq": bq_d[h],
                "bk": bk_d[h],
            }
        )
    return in_maps


def _gather(results, bv, bo, Wo):
    crow = (bv.astype(np.float32) @ Wo.T.astype(np.float32) + bo).astype(np.float32)
    out = np.empty((B, T, D), np.float32)
    for b in range(B):
        # two head-half partials per batch; sum them
        y0 = results[2 * b]["yT"]  # [P, CC, TQ] f32 = Y^T partial chunks
        y1 = results[2 * b + 1]["yT"]
        y = (y0 + y1).transpose(1, 0, 2).reshape(D, TQ).T
        out[b] = y + crow
    return out


def _run(inputs, trace=False, **kwargs):
    inputs = {k: np.asarray(v) for k, v in inputs.items()}
    nc = _get_nc()
    in_maps = _prep_in_maps(**inputs)
    res = bass_utils.run_bass_kernel_spmd(
        nc, in_maps, core_ids=list(range(N_CORES)), trace=trace, **kwargs
    )
    out = _gather(res.results, inputs["bv"], inputs["bo"], inputs["Wo"])
    return out, res


def kernel(**inputs):
    out, _ = _run(inputs, trace=False)
    return out
